# revision 1
# baseline (speedup 1.0000x reference)
"""Bass/Trainium2 kernel for nn_AlexNetOWT_BN (binarized AlexNet, batch 64).

Strategy
--------
Data-parallel convolutions (8 images per core, conv weights replicated as
sign() values in fp8e4 -> exact integer arithmetic in fp32 PSUM), conv1 in a
hi/lo integer-split fp16 pair (exact PE arithmetic, ~3e-6 total error which is
far inside the min |bn-output| margin of ~2.4e-5), tensor-parallel FC layers
(columns sharded 8-ways, activations exchanged via AllGather), log_softmax
computed redundantly on every core; core 0's [64,1000] output is returned.
"""

import os

import numpy as np
import ml_dtypes

NC = 8      # cores
B = 64      # batch
BC = 8      # images per core

F8NP = ml_dtypes.float8_e4m3fn
EPS = 1e-5

# conv layer tables (layers 2..5)
#   kh: kernel, Sp: padded input spatial, So: conv output spatial,
#   ci_t: number of 128-row ci tiles, co_t: output-channel tile widths
L2 = dict(name="c2", ci=192, co=576, kh=5, Sp=31, So=27, ci_t=2, co_t=[128, 128, 128, 128, 64], pool=True)
L3 = dict(name="c3", ci=576, co=1152, kh=3, Sp=15, So=13, ci_t=5, co_t=[128] * 9, pool=False)
L4 = dict(name="c4", ci=1152, co=768, kh=3, Sp=15, So=13, ci_t=9, co_t=[128] * 6, pool=False)
L5 = dict(name="c5", ci=768, co=256, kh=3, Sp=15, So=13, ci_t=6, co_t=[128] * 2, pool=True)


# ----------------------------------------------------------------------------
# host-side preprocessing
# ----------------------------------------------------------------------------

def _sgn(x):
    return np.sign(x).astype(np.float32)


def _conv_w_pack(w):
    """[CO,CI,kh,kw] -> [kh*kw*ci_t, 128, CO] fp8 (sign values, ci zero-padded)."""
    co, ci, kh, kw = w.shape
    arr = _sgn(w).transpose(2, 3, 1, 0).reshape(kh * kw, ci, co)
    n_ci = (ci + 127) // 128
    pad = np.zeros((kh * kw, n_ci * 128, co), np.float32)
    pad[:, :ci] = arr
    return np.ascontiguousarray(pad.reshape(kh * kw * n_ci, 128, co)).astype(F8NP)


def _im2col_c1(v):
    """[B,3,224,224] -> [384, B, 3025] fp16 (K order (ci,ky,kx), zero-padded)."""
    vp = np.pad(v, ((0, 0), (0, 0), (2, 2), (2, 2)))
    s = vp.strides
    w = np.lib.stride_tricks.as_strided(
        vp, (B, 3, 11, 11, 55, 55), (s[0], s[1], s[2], s[3], s[2] * 4, s[3] * 4))
    w = w.reshape(B, 363, 3025).transpose(1, 0, 2)
    out = np.zeros((384, B, 3025), np.float16)
    out[:363] = w
    return out


def _bn_params(cb, g, bb, m, v):
    inv = (g.astype(np.float32) / np.sqrt(v.astype(np.float32) + EPS)).astype(np.float32)
    shift = (bb.astype(np.float32) - m.astype(np.float32) * inv).astype(np.float32)
    return np.ascontiguousarray(np.stack([cb.astype(np.float32), inv, shift], 1))


def host_prep(inputs):
    d = inputs
    x = np.asarray(d["x"], np.float32)

    hi = np.rint(x * 256.0).astype(np.float32)
    assert np.abs(hi).max() <= 2040.0
    lo = np.rint((x - hi / 256.0) * float(2 ** 20)).astype(np.float32)
    assert np.abs(lo).max() <= 2048.0
    hi_c = _im2col_c1(hi)
    lo_c = _im2col_c1(lo)

    w1 = np.zeros((384, 192), np.float16)
    w1[:363] = _sgn(d["cw1"]).reshape(192, 363).T
    w1p = np.ascontiguousarray(w1.reshape(3, 128, 192))

    w2p = _conv_w_pack(d["cw2"])
    w3p = _conv_w_pack(d["cw3"])
    w4p = _conv_w_pack(d["cw4"])
    w5p = _conv_w_pack(d["cw5"])

    p1 = _bn_params(d["cb1"], d["g1"], d["bb1"], d["m1"], d["v1"])
    p2 = _bn_params(d["cb2"], d["g2"], d["bb2"], d["m2"], d["v2"])
    p3 = _bn_params(d["cb3"], d["g3"], d["bb3"], d["m3"], d["v3"])
    p4 = _bn_params(d["cb4"], d["g4"], d["bb4"], d["m4"], d["v4"])
    p5 = _bn_params(d["cb5"], d["g5"], d["bb5"], d["m5"], d["v5"])
    p6 = _bn_params(d["lb1"], d["g6"], d["bb6"], d["m6"], d["v6"])
    p7 = _bn_params(d["lb2"], d["g7"], d["bb7"], d["m7"], d["v7"])
    p8 = _bn_params(d["lb3"], d["g8"], d["bb8"], d["m8"], d["v8"])

    l1t = np.ascontiguousarray(_sgn(d["lw1"]).T).astype(ml_dtypes.bfloat16)   # [9216, 4096]
    l2t = np.ascontiguousarray(_sgn(d["lw2"]).T).astype(ml_dtypes.bfloat16)   # [4096, 4096]
    l3t = np.ascontiguousarray(_sgn(d["lw3"]).T).astype(ml_dtypes.bfloat16)   # [4096, 1000]

    in_maps = []
    for r in range(NC):
        sl = slice(r * BC, (r + 1) * BC)
        m = dict(
            rh=np.ascontiguousarray(hi_c[:, sl]).reshape(384, BC * 3025).astype(np.float16),
            rl=np.ascontiguousarray(lo_c[:, sl]).reshape(384, BC * 3025).astype(np.float16),
            w1p=w1p, w2p=w2p, w3p=w3p, w4p=w4p, w5p=w5p,
            p1=p1, p2=p2, p3=p3, p4=p4, p5=p5,
            p6=np.ascontiguousarray(p6[r * 512:(r + 1) * 512]),
            p7=np.ascontiguousarray(p7[r * 512:(r + 1) * 512]),
            p8=np.ascontiguousarray(p8[r * 125:(r + 1) * 125]),
            l1t=np.ascontiguousarray(l1t[:, r * 512:(r + 1) * 512]),
            l2t=np.ascontiguousarray(l2t[:, r * 512:(r + 1) * 512]),
            l3t=np.ascontiguousarray(l3t[:, r * 125:(r + 1) * 125]),
        )
        in_maps.append(m)
    return in_maps


# ----------------------------------------------------------------------------
# device program
# ----------------------------------------------------------------------------

_CACHE = {}


def build_nc(dump=(), single=False, reps=1):
    import concourse.bass as bass  # noqa: F401
    import concourse.mybir as mybir
    import concourse.tile as tile
    from concourse import bacc
    from concourse.masks import make_identity
    from contextlib import ExitStack

    dt = mybir.dt
    AF = mybir.ActivationFunctionType
    ALU = mybir.AluOpType

    nc = bacc.Bacc(num_devices=NC)

    # ---- I/O ----
    rh_t = nc.dram_tensor("rh", [384, BC * 3025], dt.float16, kind="ExternalInput")
    rl_t = nc.dram_tensor("rl", [384, BC * 3025], dt.float16, kind="ExternalInput")
    w1_t = nc.dram_tensor("w1p", [3, 128, 192], dt.float16, kind="ExternalInput")
    wts = {}
    for L in (L2, L3, L4, L5):
        kt = L["kh"] ** 2 * L["ci_t"]
        wts[L["name"]] = nc.dram_tensor(
            "w%sp" % L["name"][1], [kt, 128, L["co"]], dt.float8e4, kind="ExternalInput")
    pt = {}
    for i, c in zip(range(1, 9), (192, 576, 1152, 768, 256, 512, 512, 125)):
        pt[i] = nc.dram_tensor("p%d" % i, [c, 3], dt.float32, kind="ExternalInput")
    l1_t = nc.dram_tensor("l1t", [9216, 512], dt.bfloat16, kind="ExternalInput")
    l2_t = nc.dram_tensor("l2t", [4096, 512], dt.bfloat16, kind="ExternalInput")
    l3_t = nc.dram_tensor("l3t", [4096, 125], dt.bfloat16, kind="ExternalInput")
    out_t = nc.dram_tensor("out", [64, 1000], dt.float32, kind="ExternalOutput")

    dump_t = {}
    for name, shape in dict(
        a1=[192, BC, 31, 31], a2=[640, BC, 15, 15], a3=[1152, BC, 15, 15],
        a4=[768, BC, 15, 15], a5=[256, BC, 36], c1=[192, 3025],
        a6=[512, 64], lg=[1000, 64],
    ).items():
        if name in dump:
            dump_t[name] = nc.dram_tensor("d_" + name, shape, dt.float32, kind="ExternalOutput")

    RG = [list(range(NC))]

    def allgather(blk, G, rows):
        if single:
            nc.gpsimd.dma_start(G[0:rows], blk[:])
        else:
            nc.gpsimd.collective_compute("AllGather", mybir.AluOpType.bypass, replica_groups=RG,
                                         ins=[blk[:].opt()], outs=[G[:].opt()])

    def dump_tiles(name, tiles, nch, free):
        """DMA a list of [128, BC, free...] tiles (fp32-convertible) to a dump output."""
        dst = dump_t[name]
        for i, t in enumerate(tiles):
            p0 = i * 128
            pw = min(128, nch - p0)
            if pw <= 0:
                break
            for b in range(BC):
                f32 = dpool.tile([128] + free[1:], dt.float32, tag="dumpcvt", name="dcv")
                nc.vector.tensor_copy(f32[:pw], t[0:pw, b])
                nc.sync.dma_start(dst[p0:p0 + pw, b], f32[0:pw])

    with tile.TileContext(nc) as tc:
        stacks = {}

        cur_pfx = [""]

        def open_pool(key, **kw):
            key = cur_pfx[0] + key
            s = ExitStack()
            p = s.enter_context(tc.tile_pool(name=key, **kw))
            stacks[key] = s
            return p

        def close_pool(key):
            stacks.pop(cur_pfx[0] + key).close()

        dpool = open_pool("dump", bufs=2) if dump else None

        # ------------------------------------------------------------------
        # stage 1: conv1 (fp16 hi/lo) -> pool -> bn -> sign -> a1 (fp8)
        # ------------------------------------------------------------------
        for _rep in range(reps):
            _pfx = "r%d_" % _rep if reps > 1 else ""
            cur_pfx[0] = _pfx
            blk1 = nc.dram_tensor(_pfx + "blk1", [BC, 9216], dt.bfloat16)
            G1 = nc.dram_tensor(_pfx + "G1", [64, 9216], dt.bfloat16)
            blk2 = nc.dram_tensor(_pfx + "blk2", [512, 64], dt.bfloat16)
            G2 = nc.dram_tensor(_pfx + "G2", [4096, 64], dt.bfloat16)
            blk3 = nc.dram_tensor(_pfx + "blk3", [512, 64], dt.bfloat16)
            G3 = nc.dram_tensor(_pfx + "G3", [4096, 64], dt.bfloat16)
            blk4 = nc.dram_tensor(_pfx + "blk4", [125, 64], dt.float32)
            G4 = nc.dram_tensor(_pfx + "G4", [1000, 64], dt.float32)
            ppar = open_pool("par", bufs=1)
            pa1 = open_pool("a1", bufs=1)
            pr1 = open_pool("r1", bufs=2)
            pw1 = open_pool("w1", bufs=1)
            pc1 = open_pool("c1", bufs=2)
            pps1 = open_pool("ps1", bufs=3, space="PSUM")

            w1sb = pw1.tile([128, 3, 192], dt.float16)
            nc.sync.dma_start(w1sb[:], w1_t[:].rearrange("k p c -> p k c"))

            a1t = [pa1.tile([128, BC, 31, 31], dt.float8e4, tag="a1_%d" % i, name="a1_%d" % i) for i in range(2)]
            for t in a1t:
                nc.vector.memset(t[:], 0.0)
            par1s = []
            for m, (m0, mw) in enumerate(((0, 128), (128, 64))):
                par1 = ppar.tile([128, 3], dt.float32, tag="par1_%d" % m)
                nc.sync.dma_start(par1[0:mw], pt[1][m0:m0 + mw, :])
                par1s.append(par1)

            rhv = rh_t[:].rearrange("(kt p) n -> p kt n", p=128)
            rlv = rl_t[:].rearrange("(kt p) n -> p kt n", p=128)
            NT1 = [(i * 512, min(512, 3025 - i * 512)) for i in range(6)]
            for b in range(BC):
                rht = pr1.tile([128, 3, 3025], dt.float16, tag="rh")
                nc.gpsimd.dma_start(rht[:], rhv[:, :, b * 3025:(b + 1) * 3025])
                rlt = pr1.tile([128, 3, 3025], dt.float16, tag="rl")
                nc.gpsimd.dma_start(rlt[:], rlv[:, :, b * 3025:(b + 1) * 3025])
                c1i = [pc1.tile([128, 3025], dt.float32, tag="c1_%d" % m, name="c1_%d" % m) for m in range(2)]
                for m, (m0, mw) in enumerate(((0, 128), (128, 64))):
                    par1 = par1s[m]
                    for n0, nn in NT1:
                        ph = pps1.tile([128, 512], dt.float32, tag="ph")
                        pl = pps1.tile([128, 512], dt.float32, tag="pl")
                        for kt in range(3):
                            nc.tensor.matmul(ph[0:mw, 0:nn], w1sb[:, kt, m0:m0 + mw],
                                             rht[:, kt, n0:n0 + nn], start=kt == 0, stop=kt == 2)
                        for kt in range(3):
                            nc.tensor.matmul(pl[0:mw, 0:nn], w1sb[:, kt, m0:m0 + mw],
                                             rlt[:, kt, n0:n0 + nn], start=kt == 0, stop=kt == 2)
                        tlo = pc1.tile([128, 512], dt.float32, tag="tlo")
                        nc.vector.tensor_scalar(tlo[0:mw, 0:nn], pl[0:mw, 0:nn],
                                                float(2 ** -20), par1[0:mw, 0:1], ALU.mult, ALU.add)
                        nc.vector.scalar_tensor_tensor(c1i[m][0:mw, n0:n0 + nn], ph[0:mw, 0:nn],
                                                       float(2 ** -8), tlo[0:mw, 0:nn], ALU.mult, ALU.add)
                    # maxpool 55->27 then bn+sign
                    v = c1i[m][0:mw].rearrange("p (y x) -> p y x", x=55)
                    ty = pc1.tile([128, 27, 55], dt.float32, tag="ty1")
                    nc.vector.tensor_tensor(ty[0:mw], v[:, 0:53:2, :], v[:, 1:54:2, :], ALU.max)
                    nc.vector.tensor_tensor(ty[0:mw], ty[0:mw], v[:, 2:55:2, :], ALU.max)
                    pld = pc1.tile([128, 27, 27], dt.float32, tag="pl1")
                    nc.vector.tensor_tensor(pld[0:mw], ty[0:mw, :, 0:53:2], ty[0:mw, :, 1:54:2], ALU.max)
                    nc.vector.tensor_tensor(pld[0:mw], pld[0:mw], ty[0:mw, :, 2:55:2], ALU.max)
                    nc.scalar.activation(a1t[m][0:mw, b, 2:29, 2:29], pld[0:mw], AF.Sign,
                                         bias=par1[0:mw, 2:3], scale=par1[0:mw, 1:2])
                    if "c1" in dump_t and b == 0:
                        nc.sync.dma_start(dump_t["c1"][m0:m0 + mw], c1i[m][0:mw])
            close_pool("ps1"); close_pool("c1"); close_pool("w1"); close_pool("r1")
            if "a1" in dump_t:
                dump_tiles("a1", a1t, 192, [BC, 31, 31])

            # ------------------------------------------------------------------
            # conv layers 2..5
            # ------------------------------------------------------------------
            def conv_layer(L, idx, ain, pool_key_prev):
                kh, Sp, So, ci_t, co_t = L["kh"], L["Sp"], L["So"], L["ci_t"], L["co_t"]
                name = L["name"]
                KT = kh * kh * ci_t
                # output tiles (persistent pool, opened before working pools)
                if name == "c5":
                    pa = open_pool("a5", bufs=1)
                    aout = [pa.tile([128, BC, 36], dt.bfloat16, tag="a5_%d" % i, name="a5_%d" % i) for i in range(2)]
                else:
                    n_out_t = (sum(co_t) + 127) // 128
                    pa = open_pool("a" + str(idx), bufs=1)
                    aout = [pa.tile([128, BC, 15, 15], dt.float8e4, tag="a%d_%d" % (idx, i), name="a%d_%d" % (idx, i))
                            for i in range(n_out_t)]
                    for t in aout:
                        nc.vector.memset(t[:], 0.0)
                pw = open_pool("w" + name, bufs=2)
                pc = open_pool("cc" + name, bufs=4)
                pps = open_pool("ps" + name, bufs=6, space="PSUM")
                wv = wts[name][:].rearrange("k p c -> p k c")

                co0 = 0
                for mi, mw in enumerate(co_t):
                    wsb = pw.tile([128, KT, 128], dt.float8e4, tag="w" + name)
                    nc.sync.dma_start(wsb[:, :, 0:mw], wv[:, :, co0:co0 + mw])
                    part = pw.tile([128, 3], dt.float32, tag="par" + name)
                    nc.sync.dma_start(part[0:mw], pt[idx][co0:co0 + mw, :])
                    cb = part[0:mw, 0:1]
                    inv = part[0:mw, 1:2]
                    shf = part[0:mw, 2:3]
                    oc = co0 // 128
                    op = co0 % 128
                    if name == "c2":
                        for b in range(BC):
                            c2 = pc.tile([128, 27, 27], dt.float32, tag="img" + name)
                            for ys, yn in ((0, 18), (18, 9)):
                                ps = pps.tile([128, 18, 27], dt.float32, tag="ps" + name)
                                k = 0
                                for off in range(25):
                                    ky, kx = off // 5, off % 5
                                    for ct in range(ci_t):
                                        nc.tensor.matmul(
                                            ps[0:mw, 0:yn, :], wsb[:, off * ci_t + ct, 0:mw],
                                            a1t[ct][:, b, ys + ky:ys + ky + yn, kx:kx + 27],
                                            start=k == 0, stop=k == KT - 1)
                                        k += 1
                                nc.vector.tensor_scalar(c2[0:mw, ys:ys + yn, :], ps[0:mw, 0:yn, :],
                                                        cb, None, ALU.add)
                            ty = pc.tile([128, 13, 27], dt.float32, tag="ty" + name)
                            nc.vector.tensor_tensor(ty[0:mw], c2[0:mw, 0:25:2, :], c2[0:mw, 1:26:2, :], ALU.max)
                            nc.vector.tensor_tensor(ty[0:mw], ty[0:mw], c2[0:mw, 2:27:2, :], ALU.max)
                            pld = pc.tile([128, 13, 13], dt.float32, tag="pool" + name)
                            nc.vector.tensor_tensor(pld[0:mw], ty[0:mw, :, 0:25:2], ty[0:mw, :, 1:26:2], ALU.max)
                            nc.vector.tensor_tensor(pld[0:mw], pld[0:mw], ty[0:mw, :, 2:27:2], ALU.max)
                            nc.scalar.activation(aout[oc][op:op + mw, b, 1:14, 1:14], pld[0:mw],
                                                 AF.Sign, bias=shf, scale=inv)
                    else:
                        for bp in range(BC // 2):
                            ps = pps.tile([128, 2, 13, 13], dt.float32, tag="ps" + name)
                            k = 0
                            for off in range(kh * kh):
                                ky, kx = off // kh, off % kh
                                for ct in range(ci_t):
                                    nc.tensor.matmul(
                                        ps[0:mw], wsb[:, off * ci_t + ct, 0:mw],
                                        ain[ct][:, 2 * bp:2 * bp + 2, ky:ky + 13, kx:kx + 13],
                                        start=k == 0, stop=k == KT - 1)
                                    k += 1
                            y = pc.tile([128, 2, 13, 13], dt.float32, tag="img" + name)
                            nc.vector.tensor_scalar(y[0:mw], ps[0:mw], cb, None, ALU.add)
                            if name == "c5":
                                for s in range(2):
                                    b = 2 * bp + s
                                    ty = pc.tile([128, 6, 13], dt.float32, tag="ty" + name)
                                    nc.vector.tensor_tensor(ty[0:mw], y[0:mw, s, 0:11:2, :],
                                                            y[0:mw, s, 1:12:2, :], ALU.max)
                                    nc.vector.tensor_tensor(ty[0:mw], ty[0:mw], y[0:mw, s, 2:13:2, :], ALU.max)
                                    pld = pc.tile([128, 6, 6], dt.float32, tag="pool" + name)
                                    nc.vector.tensor_tensor(pld[0:mw], ty[0:mw, :, 0:11:2],
                                                            ty[0:mw, :, 1:12:2], ALU.max)
                                    nc.vector.tensor_tensor(pld[0:mw], pld[0:mw], ty[0:mw, :, 2:13:2], ALU.max)
                                    nc.scalar.activation(
                                        aout[oc][op:op + mw, b].rearrange("p (y x) -> p y x", x=6),
                                        pld[0:mw], AF.Sign, bias=shf, scale=inv)
                            else:
                                nc.scalar.activation(aout[oc][op:op + mw, 2 * bp:2 * bp + 2, 1:14, 1:14],
                                                     y[0:mw], AF.Sign, bias=shf, scale=inv)
                    co0 += mw
                close_pool("ps" + name); close_pool("cc" + name); close_pool("w" + name)
                return aout

            a2t = conv_layer(L2, 2, a1t, "a1")
            if "a2" in dump_t:
                dump_tiles("a2", a2t, 640, [BC, 15, 15])
            a3t = conv_layer(L3, 3, a2t, "a2")
            if "a3" in dump_t:
                dump_tiles("a3", a3t, 1152, [BC, 15, 15])
            a4t = conv_layer(L4, 4, a3t, "a3")
            if "a4" in dump_t:
                dump_tiles("a4", a4t, 768, [BC, 15, 15])
            a5t = conv_layer(L5, 5, a4t, "a4")
            if "a5" in dump_t:
                dump_tiles("a5", a5t, 256, [BC, 36])

            # ------------------------------------------------------------------
            # FC layers (tensor parallel) + log_softmax
            # ------------------------------------------------------------------
            pfc = open_pool("fc", bufs=2)
            pfw = open_pool("fw", bufs=2)
            ppsf = open_pool("psf", bufs=4, space="PSUM")

            # a5 -> blk1 [8, 9216] (b-major rows)
            b1v = blk1[:].rearrange("b (c hw) -> c b hw", hw=36)
            for ch in range(2):
                nc.gpsimd.dma_start(b1v[ch * 128:(ch + 1) * 128], a5t[ch][:])
            allgather(blk1, G1, BC)

            # FC1: rhs via transpose DMAs
            r1t = pfc.tile([128, 72, 64], dt.bfloat16, tag="r1")
            for kc in range(72):
                nc.sync.dma_start(r1t[:, kc, :], G1[0:64, kc * 128:(kc + 1) * 128], transpose=True)
            l1v = l1_t[:].rearrange("(kt p) c -> p kt c", p=128)
            a6s = []
            for m in range(4):
                wsb = pfw.tile([128, 72, 128], dt.bfloat16, tag="l1w")
                nc.sync.dma_start(wsb[:], l1v[:, :, m * 128:(m + 1) * 128])
                par6 = pfw.tile([128, 3], dt.float32, tag="par6")
                nc.sync.dma_start(par6[:], pt[6][m * 128:(m + 1) * 128, :])
                ps = ppsf.tile([128, 64], dt.float32, tag="psf")
                for kc in range(72):
                    nc.tensor.matmul(ps[:], wsb[:, kc, :], r1t[:, kc, :], start=kc == 0, stop=kc == 71)
                y = pfc.tile([128, 64], dt.float32, tag="y6")
                nc.vector.tensor_scalar(y[:], ps[:], par6[:, 0:1], None, ALU.add)
                a6 = pfc.tile([128, 64], dt.bfloat16, tag="a6")
                nc.scalar.activation(a6[:], y[:], AF.Sign,
                                     bias=par6[:, 2:3],
                                     scale=par6[:, 1:2])
                a6s.append(a6)
                nc.gpsimd.dma_start(blk2[m * 128:(m + 1) * 128, :], a6[:])
            allgather(blk2, G2, 512)
            if "a6" in dump_t:
                for m in range(4):
                    f32 = dpool.tile([128, 64], dt.float32, tag="dumpcvt2")
                    nc.vector.tensor_copy(f32[:], a6s[m][:])
                    nc.sync.dma_start(dump_t["a6"][m * 128:(m + 1) * 128], f32[:])

            # FC2
            r2t = pfc.tile([128, 32, 64], dt.bfloat16, tag="r2")
            nc.sync.dma_start(r2t[:], G2[:].rearrange("(kt p) b -> p kt b", p=128))
            l2v = l2_t[:].rearrange("(kt p) c -> p kt c", p=128)
            for m in range(4):
                wsb = pfw.tile([128, 32, 128], dt.bfloat16, tag="l2w")
                nc.sync.dma_start(wsb[:], l2v[:, :, m * 128:(m + 1) * 128])
                par7 = pfw.tile([128, 3], dt.float32, tag="par6")
                nc.sync.dma_start(par7[:], pt[7][m * 128:(m + 1) * 128, :])
                ps = ppsf.tile([128, 64], dt.float32, tag="psf")
                for kc in range(32):
                    nc.tensor.matmul(ps[:], wsb[:, kc, :], r2t[:, kc, :], start=kc == 0, stop=kc == 31)
                y = pfc.tile([128, 64], dt.float32, tag="y6")
                nc.vector.tensor_scalar(y[:], ps[:], par7[:, 0:1], None, ALU.add)
                a7 = pfc.tile([128, 64], dt.bfloat16, tag="a6")
                nc.scalar.activation(a7[:], y[:], AF.Sign,
                                     bias=par7[:, 2:3],
                                     scale=par7[:, 1:2])
                nc.gpsimd.dma_start(blk3[m * 128:(m + 1) * 128, :], a7[:])
            allgather(blk3, G3, 512)

            # FC3 + bn8
            r3t = pfc.tile([128, 32, 64], dt.bfloat16, tag="r2")
            nc.sync.dma_start(r3t[:], G3[:].rearrange("(kt p) b -> p kt b", p=128))
            par8 = pfw.tile([125, 3], dt.float32, tag="par8")
            nc.sync.dma_start(par8[:], pt[8][:])
            l3v = l3_t[:].rearrange("(kt p) c -> p kt c", p=128)
            w3sb = pfw.tile([128, 32, 125], dt.bfloat16, tag="l3w")
            nc.sync.dma_start(w3sb[:], l3v[:])
            ps = ppsf.tile([128, 64], dt.float32, tag="psf")
            for kc in range(32):
                nc.tensor.matmul(ps[0:125, :], w3sb[:, kc, :], r3t[:, kc, :], start=kc == 0, stop=kc == 31)
            y8 = pfc.tile([125, 64], dt.float32, tag="y8")
            nc.vector.tensor_scalar(y8[:], ps[0:125, :], par8[:, 0:1], None, ALU.add)
            z8 = pfc.tile([125, 64], dt.float32, tag="z8")
            nc.vector.tensor_scalar(z8[:], y8[:], par8[:, 1:2], par8[:, 2:3], ALU.mult, ALU.add)
            nc.gpsimd.dma_start(blk4[:], z8[:])
            allgather(blk4, G4, 125)

            # gather logits -> [64, 1000] via PE transpose, then log_softmax
            idt = ppar.tile([128, 128], dt.float32, tag="ident")
            make_identity(nc, idt[:])
            lg = pfc.tile([64, 1000], dt.float32, tag="lg")
            for r in range(8):
                lt = pfc.tile([125, 64], dt.float32, tag="lt")
                nc.sync.dma_start(lt[:], G4[r * 125:(r + 1) * 125, :])
                tp = ppsf.tile([64, 128], dt.float32, tag="tp")
                nc.tensor.transpose(tp[0:64, 0:125], lt[:], idt[0:125, 0:125])
                nc.vector.tensor_copy(lg[:, r * 125:(r + 1) * 125], tp[0:64, 0:125])
            if "lg" in dump_t:
                for r in range(8):
                    nc.sync.dma_start(dump_t["lg"][r * 125:(r + 1) * 125], G4[r * 125:(r + 1) * 125, :])
            mx = pfc.tile([64, 1], dt.float32, tag="mx")
            nc.vector.reduce_max(mx[:], lg[:], axis=mybir.AxisListType.X)
            sh = pfc.tile([64, 1000], dt.float32, tag="sh")
            nc.vector.tensor_scalar(sh[:], lg[:], mx[:], None, ALU.subtract)
            ex = pfc.tile([64, 1000], dt.float32, tag="ex")
            nc.scalar.activation(ex[:], sh[:], AF.Exp)
            sm = pfc.tile([64, 1], dt.float32, tag="sm")
            nc.vector.reduce_sum(sm[:], ex[:], axis=mybir.AxisListType.X)
            ls = pfc.tile([64, 1], dt.float32, tag="ls")
            nc.scalar.activation(ls[:], sm[:], AF.Ln)
            osb = pfc.tile([64, 1000], dt.float32, tag="osb")
            nc.vector.tensor_scalar(osb[:], sh[:], ls[:], None, ALU.subtract)
            nc.sync.dma_start(out_t[:], osb[:])

            for k in reversed(list(stacks)):
                stacks.pop(k).close()

    nc.finalize()
    return nc


def get_nc(dump=()):
    key = tuple(sorted(dump))
    if key not in _CACHE:
        _CACHE[key] = build_nc(dump)
    return _CACHE[key]


def kernel(**inputs):
    from concourse.bass_utils import run_bass_kernel_spmd

    dump = tuple(d for d in os.environ.get("BASS_DUMP", "").split(",") if d)
    nc = get_nc(dump)
    in_maps = host_prep(inputs)
    res = run_bass_kernel_spmd(nc, in_maps, core_ids=list(range(NC)))
    out = np.asarray(res.results[0]["out"], np.float32)
    if dump:
        kernel.dumps = [{k: v for k, v in r.items() if k.startswith("d_")} for r in res.results]
    return out



# revision 5
# speedup vs baseline: 1.1216x; 1.1216x over previous
"""Bass/Trainium2 kernel for nn_AlexNetOWT_BN (binarized AlexNet, batch 64).

Strategy
--------
Data-parallel convolutions (8 images per core, conv weights replicated as
sign() values in fp8e4 -> exact integer arithmetic in fp32 PSUM), conv1 in a
hi/lo integer-split fp16 pair (exact PE arithmetic, ~3e-6 total error which is
far inside the min |bn-output| margin of ~2.4e-5), conv2 in fp8 DoubleRow
(K=256 per matmul: both ci tiles fused, halving PE stream cycles),
tensor-parallel FC layers (columns sharded 8-ways, activations exchanged via
AllGather), log_softmax computed redundantly on every core; core 0's [64,1000]
output is returned.

Conv biases and FC biases are folded into the BN shift on the host
(sign(inv*(x+cb)+shf) == sign(inv*x + (inv*cb+shf)), and max-pool commutes
with the per-channel constant add), so every post-matmul drain is a single
activation (or pool chain) reading PSUM directly.
"""

import os

import numpy as np
import ml_dtypes

NC = 8      # cores
B = 64      # batch
BC = 8      # images per core

F8NP = ml_dtypes.float8_e4m3fn
EPS = 1e-5

# conv layer tables (layers 3..5)
#   kh: kernel, Sp: padded input spatial, So: conv output spatial,
#   ci_t: number of 128-row ci tiles, co_t: output-channel tile widths
L3 = dict(name="c3", ci=576, co=1152, kh=3, Sp=15, So=13, ci_t=5, co_t=[128] * 9, pool=False)
L4 = dict(name="c4", ci=1152, co=768, kh=3, Sp=15, So=13, ci_t=9, co_t=[128] * 6, pool=False)
L5 = dict(name="c5", ci=768, co=256, kh=3, Sp=15, So=13, ci_t=6, co_t=[128] * 2, pool=True)

CO2_T = [128, 128, 128, 128, 64]   # conv2 output-channel tiles


# ----------------------------------------------------------------------------
# host-side preprocessing
# ----------------------------------------------------------------------------

def _sgn(x):
    return np.sign(x).astype(np.float32)


def _conv_w_pack(w):
    """[CO,CI,kh,kw] -> [kh*kw*ci_t, 128, CO] fp8 (sign values, ci zero-padded)."""
    co, ci, kh, kw = w.shape
    arr = _sgn(w).transpose(2, 3, 1, 0).reshape(kh * kw, ci, co)
    n_ci = (ci + 127) // 128
    pad = np.zeros((kh * kw, n_ci * 128, co), np.float32)
    pad[:, :ci] = arr
    return np.ascontiguousarray(pad.reshape(kh * kw * n_ci, 128, co)).astype(F8NP)


def _conv2_w_pack(w):
    """[576,192,5,5] -> [5 mi, 128 p, 25 off, 2 j, 128 c] fp8 DoubleRow layout."""
    sw = _sgn(w)  # [576, 192, 5, 5]
    out = np.zeros((5, 128, 25, 2, 128), np.float32)
    for mi, mw in enumerate(CO2_T):
        co0 = mi * 128
        for j in range(2):
            ci0, ciw = j * 128, min(128, 192 - j * 128)
            # [mw, ciw, 5, 5] -> [ciw(p), 25(off), mw(c)]
            blk = sw[co0:co0 + mw, ci0:ci0 + ciw].reshape(mw, ciw, 25).transpose(1, 2, 0)
            out[mi, :ciw, :, j, :mw] = blk
    return np.ascontiguousarray(out.reshape(5, 128, 25 * 2 * 128)).astype(F8NP)


def _im2col_c1(v):
    """[B,3,224,224] -> [384, B, 3025] fp16 (K order (ci,ky,kx), zero-padded)."""
    vp = np.pad(v, ((0, 0), (0, 0), (2, 2), (2, 2)))
    s = vp.strides
    w = np.lib.stride_tricks.as_strided(
        vp, (B, 3, 11, 11, 55, 55), (s[0], s[1], s[2], s[3], s[2] * 4, s[3] * 4))
    w = w.reshape(B, 363, 3025).transpose(1, 0, 2)
    out = np.zeros((384, B, 3025), np.float16)
    out[:363] = w
    return out


def _bn_params(cb, g, bb, m, v):
    """[inv, shf'] with the conv/linear bias folded into the shift."""
    inv = (g.astype(np.float32) / np.sqrt(v.astype(np.float32) + EPS)).astype(np.float32)
    shift = (bb.astype(np.float32) - m.astype(np.float32) * inv
             + cb.astype(np.float32) * inv).astype(np.float32)
    return np.ascontiguousarray(np.stack([inv, shift], 1))


def host_prep(inputs):
    d = inputs
    x = np.asarray(d["x"], np.float32)

    hi = np.rint(x * 256.0).astype(np.float32)
    assert np.abs(hi).max() <= 2040.0
    lo = np.rint((x - hi / 256.0) * float(2 ** 20)).astype(np.float32)
    assert np.abs(lo).max() <= 2048.0
    hi_c = _im2col_c1(hi)
    lo_c = _im2col_c1(lo)

    w1 = np.zeros((384, 192), np.float16)
    w1[:363] = _sgn(d["cw1"]).reshape(192, 363).T
    w1p = np.ascontiguousarray(w1.reshape(3, 128, 192))

    w2p = _conv2_w_pack(d["cw2"])
    w3p = _conv_w_pack(d["cw3"])
    w4p = _conv_w_pack(d["cw4"])
    w5p = _conv_w_pack(d["cw5"])

    p1 = _bn_params(d["cb1"], d["g1"], d["bb1"], d["m1"], d["v1"])
    p2 = _bn_params(d["cb2"], d["g2"], d["bb2"], d["m2"], d["v2"])
    p3 = _bn_params(d["cb3"], d["g3"], d["bb3"], d["m3"], d["v3"])
    p4 = _bn_params(d["cb4"], d["g4"], d["bb4"], d["m4"], d["v4"])
    p5 = _bn_params(d["cb5"], d["g5"], d["bb5"], d["m5"], d["v5"])
    p6 = _bn_params(d["lb1"], d["g6"], d["bb6"], d["m6"], d["v6"])
    p7 = _bn_params(d["lb2"], d["g7"], d["bb7"], d["m7"], d["v7"])
    p8 = _bn_params(d["lb3"], d["g8"], d["bb8"], d["m8"], d["v8"])

    l1t = np.ascontiguousarray(_sgn(d["lw1"]).T).astype(ml_dtypes.bfloat16)   # [9216, 4096]
    l2t = np.ascontiguousarray(_sgn(d["lw2"]).T).astype(ml_dtypes.bfloat16)   # [4096, 4096]
    l3t = np.ascontiguousarray(_sgn(d["lw3"]).T).astype(ml_dtypes.bfloat16)   # [4096, 1000]

    in_maps = []
    for r in range(NC):
        sl = slice(r * BC, (r + 1) * BC)
        m = dict(
            rh=np.ascontiguousarray(hi_c[:, sl]).reshape(384, BC * 3025).astype(np.float16),
            rl=np.ascontiguousarray(lo_c[:, sl]).reshape(384, BC * 3025).astype(np.float16),
            w1p=w1p, w2p=w2p, w3p=w3p, w4p=w4p, w5p=w5p,
            p1=p1, p2=p2, p3=p3, p4=p4, p5=p5,
            p6=np.ascontiguousarray(p6[r * 512:(r + 1) * 512]),
            p7=np.ascontiguousarray(p7[r * 512:(r + 1) * 512]),
            p8=np.ascontiguousarray(p8[r * 125:(r + 1) * 125]),
            l1t=np.ascontiguousarray(l1t[:, r * 512:(r + 1) * 512]),
            l2t=np.ascontiguousarray(l2t[:, r * 512:(r + 1) * 512]),
            l3t=np.ascontiguousarray(l3t[:, r * 125:(r + 1) * 125]),
        )
        in_maps.append(m)
    return in_maps


# ----------------------------------------------------------------------------
# device program
# ----------------------------------------------------------------------------

_CACHE = {}


def build_nc(dump=(), single=False, reps=1):
    import concourse.bass as bass  # noqa: F401
    import concourse.mybir as mybir
    import concourse.tile as tile
    from concourse import bacc
    from concourse.masks import make_identity
    from contextlib import ExitStack

    dt = mybir.dt
    AF = mybir.ActivationFunctionType
    ALU = mybir.AluOpType
    DR = mybir.MatmulPerfMode.DoubleRow

    nc = bacc.Bacc(num_devices=NC)

    # ---- I/O ----
    rh_t = nc.dram_tensor("rh", [384, BC * 3025], dt.float16, kind="ExternalInput")
    rl_t = nc.dram_tensor("rl", [384, BC * 3025], dt.float16, kind="ExternalInput")
    w1_t = nc.dram_tensor("w1p", [3, 128, 192], dt.float16, kind="ExternalInput")
    w2_t = nc.dram_tensor("w2p", [5, 128, 25 * 2 * 128], dt.float8e4, kind="ExternalInput")
    wts = {}
    for L in (L3, L4, L5):
        kt = L["kh"] ** 2 * L["ci_t"]
        wts[L["name"]] = nc.dram_tensor(
            "w%sp" % L["name"][1], [kt, 128, L["co"]], dt.float8e4, kind="ExternalInput")
    pt = {}
    for i, c in zip(range(1, 9), (192, 576, 1152, 768, 256, 512, 512, 125)):
        pt[i] = nc.dram_tensor("p%d" % i, [c, 2], dt.float32, kind="ExternalInput")
    l1_t = nc.dram_tensor("l1t", [9216, 512], dt.bfloat16, kind="ExternalInput")
    l2_t = nc.dram_tensor("l2t", [4096, 512], dt.bfloat16, kind="ExternalInput")
    l3_t = nc.dram_tensor("l3t", [4096, 125], dt.bfloat16, kind="ExternalInput")
    out_t = nc.dram_tensor("out", [64, 1000], dt.float32, kind="ExternalOutput")

    RG = [list(range(NC))]

    def allgather(blk, G, rows):
        if single:
            nc.gpsimd.dma_start(G[0:rows], blk[:])
        else:
            nc.gpsimd.collective_compute("AllGather", mybir.AluOpType.bypass, replica_groups=RG,
                                         ins=[blk[:].opt()], outs=[G[:].opt()])

    with tile.TileContext(nc) as tc:
        stacks = {}

        cur_pfx = [""]

        def open_pool(key, **kw):
            key = cur_pfx[0] + key
            s = ExitStack()
            p = s.enter_context(tc.tile_pool(name=key, **kw))
            stacks[key] = s
            return p

        def close_pool(key):
            stacks.pop(cur_pfx[0] + key).close()

        for _rep in range(reps):
            _pfx = "r%d_" % _rep if reps > 1 else ""
            cur_pfx[0] = _pfx
            blk1 = nc.dram_tensor(_pfx + "blk1", [BC, 9216], dt.bfloat16)
            G1 = nc.dram_tensor(_pfx + "G1", [64, 9216], dt.bfloat16)
            blk2 = nc.dram_tensor(_pfx + "blk2", [512, 64], dt.bfloat16)
            G2 = nc.dram_tensor(_pfx + "G2", [4096, 64], dt.bfloat16)
            blk3 = nc.dram_tensor(_pfx + "blk3", [512, 64], dt.bfloat16)
            G3 = nc.dram_tensor(_pfx + "G3", [4096, 64], dt.bfloat16)
            blk4 = nc.dram_tensor(_pfx + "blk4", [125, 64], dt.float32)
            G4 = nc.dram_tensor(_pfx + "G4", [1000, 64], dt.float32)

            # ------------------------------------------------------------------
            # stage 1: conv1 (fp16 hi/lo) -> pool -> bn -> sign -> a1 (fp8 DR)
            # ------------------------------------------------------------------
            ppar = open_pool("par", bufs=1)
            pa1 = open_pool("a1", bufs=1)
            pr1 = open_pool("r1", bufs=2)
            pw1 = open_pool("w1", bufs=1)
            pc1 = open_pool("c1", bufs=2)
            pps1 = open_pool("ps1", bufs=3, space="PSUM")

            w1sb = pw1.tile([128, 3, 192], dt.float16)
            nc.sync.dma_start(w1sb[:], w1_t[:].rearrange("k p c -> p k c"))

            # conv2 input, DoubleRow layout: [p, j(ci half), b, y, x]
            a1dr = pa1.tile([128, 2, BC, 31, 32], dt.float8e4, name="a1dr")
            nc.vector.memset(a1dr[:], 0.0)
            par1s = []
            for m, (m0, mw) in enumerate(((0, 128), (128, 64))):
                par1 = ppar.tile([128, 2], dt.float32, tag="par1_%d" % m)
                nc.sync.dma_start(par1[0:mw], pt[1][m0:m0 + mw, :])
                par1s.append(par1)

            rhv = rh_t[:].rearrange("(kt p) n -> p kt n", p=128)
            rlv = rl_t[:].rearrange("(kt p) n -> p kt n", p=128)
            NT1 = [(i * 512, min(512, 3025 - i * 512)) for i in range(6)]
            for b in range(BC):
                rht = pr1.tile([128, 3, 3025], dt.float16, tag="rh")
                nc.gpsimd.dma_start(rht[:], rhv[:, :, b * 3025:(b + 1) * 3025])
                rlt = pr1.tile([128, 3, 3025], dt.float16, tag="rl")
                nc.gpsimd.dma_start(rlt[:], rlv[:, :, b * 3025:(b + 1) * 3025])
                c1i = [pc1.tile([128, 3025], dt.float32, tag="c1_%d" % m, name="c1_%d" % m) for m in range(2)]
                for m, (m0, mw) in enumerate(((0, 128), (128, 64))):
                    par1 = par1s[m]
                    for n0, nn in NT1:
                        ph = pps1.tile([128, 512], dt.float32, tag="ph")
                        pl = pps1.tile([128, 512], dt.float32, tag="pl")
                        for kt in range(3):
                            nc.tensor.matmul(ph[0:mw, 0:nn], w1sb[:, kt, m0:m0 + mw],
                                             rht[:, kt, n0:n0 + nn], start=kt == 0, stop=kt == 2)
                        for kt in range(3):
                            nc.tensor.matmul(pl[0:mw, 0:nn], w1sb[:, kt, m0:m0 + mw],
                                             rlt[:, kt, n0:n0 + nn], start=kt == 0, stop=kt == 2)
                        tlo = pc1.tile([128, 512], dt.float32, tag="tlo")
                        nc.vector.tensor_scalar(tlo[0:mw, 0:nn], pl[0:mw, 0:nn],
                                                float(2 ** -20), None, ALU.mult)
                        nc.vector.scalar_tensor_tensor(c1i[m][0:mw, n0:n0 + nn], ph[0:mw, 0:nn],
                                                       float(2 ** -8), tlo[0:mw, 0:nn], ALU.mult, ALU.add)
                    # maxpool 55->27 then bn+sign
                    v = c1i[m][0:mw].rearrange("p (y x) -> p y x", x=55)
                    ty = pc1.tile([128, 27, 55], dt.float32, tag="ty1")
                    nc.vector.tensor_tensor(ty[0:mw], v[:, 0:53:2, :], v[:, 1:54:2, :], ALU.max)
                    nc.vector.tensor_tensor(ty[0:mw], ty[0:mw], v[:, 2:55:2, :], ALU.max)
                    pld = pc1.tile([128, 27, 27], dt.float32, tag="pl1")
                    nc.vector.tensor_tensor(pld[0:mw], ty[0:mw, :, 0:53:2], ty[0:mw, :, 1:54:2], ALU.max)
                    nc.vector.tensor_tensor(pld[0:mw], pld[0:mw], ty[0:mw, :, 2:55:2], ALU.max)
                    nc.scalar.activation(a1dr[0:mw, m, b, 2:29, 2:29], pld[0:mw], AF.Sign,
                                         bias=par1[0:mw, 1:2], scale=par1[0:mw, 0:1])
            close_pool("ps1"); close_pool("c1"); close_pool("w1"); close_pool("r1")

            # ------------------------------------------------------------------
            # conv2: fp8 DoubleRow (K=256), pool from PSUM, -> a2 tiles
            # ------------------------------------------------------------------
            pa2 = open_pool("a2", bufs=1)
            a2t = [pa2.tile([128, BC, 15, 15], dt.float8e4, tag="a2_%d" % i, name="a2_%d" % i)
                   for i in range(5)]
            for t in a2t:
                nc.vector.memset(t[:], 0.0)
            pw2 = open_pool("w2", bufs=2)
            pc2 = open_pool("cc2", bufs=4)
            pps2 = open_pool("ps2", bufs=1, space="PSUM")

            co0 = 0
            for mi, mw in enumerate(CO2_T):
                wsb = pw2.tile([128, 25, 2, 128], dt.float8e4, tag="w2")
                nc.sync.dma_start(wsb[:].rearrange("p k j c -> p (k j c)"), w2_t[mi])
                part = pw2.tile([128, 2], dt.float32, tag="par2")
                nc.sync.dma_start(part[0:mw], pt[2][co0:co0 + mw, :])
                inv = part[0:mw, 0:1]
                shf = part[0:mw, 1:2]
                oc = co0 // 128
                op = co0 % 128
                for g in range(2):
                    psA = [pps2.tile([128, 18, 27], dt.float32, tag="psA%d" % i, name="psA%d" % i) for i in range(4)]
                    psB = [pps2.tile([128, 9, 27], dt.float32, tag="psB%d" % i, name="psB%d" % i) for i in range(4)]
                    for off in range(25):
                        ky, kx = off // 5, off % 5
                        for i in range(4):
                            b = 4 * g + i
                            nc.tensor.matmul(psA[i][0:mw], wsb[:, off, :, 0:mw],
                                             a1dr[:, :, b, ky:ky + 18, kx:kx + 27],
                                             start=off == 0, stop=off == 24, perf_mode=DR)
                            nc.tensor.matmul(psB[i][0:mw], wsb[:, off, :, 0:mw],
                                             a1dr[:, :, b, 18 + ky:27 + ky, kx:kx + 27],
                                             start=off == 0, stop=off == 24, perf_mode=DR)
                    for i in range(4):
                        b = 4 * g + i
                        c2 = pc2.tile([128, 27, 27], dt.float32, tag="img2")
                        nc.vector.tensor_copy(c2[0:mw, 0:18], psA[i][0:mw])
                        nc.vector.tensor_copy(c2[0:mw, 18:27], psB[i][0:mw])
                        ty = pc2.tile([128, 13, 27], dt.float32, tag="ty2")
                        nc.vector.tensor_tensor(ty[0:mw], c2[0:mw, 0:25:2], c2[0:mw, 1:26:2], ALU.max)
                        nc.vector.tensor_tensor(ty[0:mw], ty[0:mw], c2[0:mw, 2:27:2], ALU.max)
                        pld = pc2.tile([128, 13, 13], dt.float32, tag="pool2")
                        nc.vector.tensor_tensor(pld[0:mw], ty[0:mw, :, 0:25:2], ty[0:mw, :, 1:26:2], ALU.max)
                        nc.vector.tensor_tensor(pld[0:mw], pld[0:mw], ty[0:mw, :, 2:27:2], ALU.max)
                        nc.scalar.activation(a2t[oc][op:op + mw, b, 1:14, 1:14], pld[0:mw],
                                             AF.Sign, bias=shf, scale=inv)
                co0 += mw
            close_pool("ps2"); close_pool("cc2"); close_pool("w2")

            # ------------------------------------------------------------------
            # conv layers 3..5 (normal fp8, drains read PSUM directly)
            # ------------------------------------------------------------------
            def conv_layer(L, idx, ain):
                kh, ci_t, co_t = L["kh"], L["ci_t"], L["co_t"]
                name = L["name"]
                KT = kh * kh * ci_t
                if name == "c5":
                    pa = open_pool("a5", bufs=1)
                    aout = [pa.tile([128, BC, 36], dt.bfloat16, tag="a5_%d" % i, name="a5_%d" % i) for i in range(2)]
                else:
                    n_out_t = (sum(co_t) + 127) // 128
                    pa = open_pool("a" + str(idx), bufs=1)
                    aout = [pa.tile([128, BC, 15, 15], dt.float8e4, tag="a%d_%d" % (idx, i), name="a%d_%d" % (idx, i))
                            for i in range(n_out_t)]
                    for t in aout:
                        nc.vector.memset(t[:], 0.0)
                pw = open_pool("w" + name, bufs=2)
                pc = open_pool("cc" + name, bufs=4)
                pps = open_pool("ps" + name, bufs=6, space="PSUM")
                wv = wts[name][:].rearrange("k p c -> p k c")

                co0 = 0
                for mi, mw in enumerate(co_t):
                    wsb = pw.tile([128, KT, 128], dt.float8e4, tag="w" + name)
                    nc.sync.dma_start(wsb[:, :, 0:mw], wv[:, :, co0:co0 + mw])
                    part = pw.tile([128, 2], dt.float32, tag="par" + name)
                    nc.sync.dma_start(part[0:mw], pt[idx][co0:co0 + mw, :])
                    inv = part[0:mw, 0:1]
                    shf = part[0:mw, 1:2]
                    oc = co0 // 128
                    op = co0 % 128
                    for bp in range(BC // 2):
                        ps = pps.tile([128, 2, 13, 13], dt.float32, tag="ps" + name)
                        k = 0
                        for off in range(kh * kh):
                            ky, kx = off // kh, off % kh
                            for ct in range(ci_t):
                                nc.tensor.matmul(
                                    ps[0:mw], wsb[:, off * ci_t + ct, 0:mw],
                                    ain[ct][:, 2 * bp:2 * bp + 2, ky:ky + 13, kx:kx + 13],
                                    start=k == 0, stop=k == KT - 1)
                                k += 1
                        if name == "c5":
                            y5 = pc.tile([128, 2, 13, 13], dt.float32, tag="img" + name)
                            nc.vector.tensor_copy(y5[0:mw], ps[0:mw])
                            for s in range(2):
                                b = 2 * bp + s
                                ty = pc.tile([128, 6, 13], dt.float32, tag="ty" + name)
                                nc.vector.tensor_tensor(ty[0:mw], y5[0:mw, s, 0:11:2, :],
                                                        y5[0:mw, s, 1:12:2, :], ALU.max)
                                nc.vector.tensor_tensor(ty[0:mw], ty[0:mw], y5[0:mw, s, 2:13:2, :], ALU.max)
                                pld = pc.tile([128, 6, 6], dt.float32, tag="pool" + name)
                                nc.vector.tensor_tensor(pld[0:mw], ty[0:mw, :, 0:11:2],
                                                        ty[0:mw, :, 1:12:2], ALU.max)
                                nc.vector.tensor_tensor(pld[0:mw], pld[0:mw], ty[0:mw, :, 2:13:2], ALU.max)
                                nc.scalar.activation(
                                    aout[oc][op:op + mw, b].rearrange("p (y x) -> p y x", x=6),
                                    pld[0:mw], AF.Sign, bias=shf, scale=inv)
                        else:
                            nc.scalar.activation(aout[oc][op:op + mw, 2 * bp:2 * bp + 2, 1:14, 1:14],
                                                 ps[0:mw], AF.Sign, bias=shf, scale=inv)
                    co0 += mw
                close_pool("ps" + name); close_pool("cc" + name); close_pool("w" + name)
                return aout

            a3t = conv_layer(L3, 3, a2t)
            a4t = conv_layer(L4, 4, a3t)
            a5t = conv_layer(L5, 5, a4t)

            # ------------------------------------------------------------------
            # FC layers (tensor parallel) + log_softmax
            # ------------------------------------------------------------------
            pfc = open_pool("fc", bufs=2)
            pfw = open_pool("fw", bufs=2)
            ppsf = open_pool("psf", bufs=4, space="PSUM")

            # a5 -> blk1 [8, 9216] (b-major rows)
            b1v = blk1[:].rearrange("b (c hw) -> c b hw", hw=36)
            for ch in range(2):
                nc.gpsimd.dma_start(b1v[ch * 128:(ch + 1) * 128], a5t[ch][:])
            allgather(blk1, G1, BC)

            # FC1: rhs via transpose DMAs
            r1t = pfc.tile([128, 72, 64], dt.bfloat16, tag="r1")
            for kc in range(72):
                nc.sync.dma_start(r1t[:, kc, :], G1[0:64, kc * 128:(kc + 1) * 128], transpose=True)
            l1v = l1_t[:].rearrange("(kt p) c -> p kt c", p=128)
            for m in range(4):
                wsb = pfw.tile([128, 72, 128], dt.bfloat16, tag="l1w")
                nc.sync.dma_start(wsb[:], l1v[:, :, m * 128:(m + 1) * 128])
                par6 = pfw.tile([128, 2], dt.float32, tag="par6")
                nc.sync.dma_start(par6[:], pt[6][m * 128:(m + 1) * 128, :])
                ps = ppsf.tile([128, 64], dt.float32, tag="psf")
                for kc in range(72):
                    nc.tensor.matmul(ps[:], wsb[:, kc, :], r1t[:, kc, :], start=kc == 0, stop=kc == 71)
                a6 = pfc.tile([128, 64], dt.bfloat16, tag="a6")
                nc.scalar.activation(a6[:], ps[:], AF.Sign,
                                     bias=par6[:, 1:2], scale=par6[:, 0:1])
                nc.gpsimd.dma_start(blk2[m * 128:(m + 1) * 128, :], a6[:])
            allgather(blk2, G2, 512)

            # FC2
            r2t = pfc.tile([128, 32, 64], dt.bfloat16, tag="r2")
            nc.sync.dma_start(r2t[:], G2[:].rearrange("(kt p) b -> p kt b", p=128))
            l2v = l2_t[:].rearrange("(kt p) c -> p kt c", p=128)
            for m in range(4):
                wsb = pfw.tile([128, 32, 128], dt.bfloat16, tag="l2w")
                nc.sync.dma_start(wsb[:], l2v[:, :, m * 128:(m + 1) * 128])
                par7 = pfw.tile([128, 2], dt.float32, tag="par6")
                nc.sync.dma_start(par7[:], pt[7][m * 128:(m + 1) * 128, :])
                ps = ppsf.tile([128, 64], dt.float32, tag="psf")
                for kc in range(32):
                    nc.tensor.matmul(ps[:], wsb[:, kc, :], r2t[:, kc, :], start=kc == 0, stop=kc == 31)
                a7 = pfc.tile([128, 64], dt.bfloat16, tag="a6")
                nc.scalar.activation(a7[:], ps[:], AF.Sign,
                                     bias=par7[:, 1:2], scale=par7[:, 0:1])
                nc.gpsimd.dma_start(blk3[m * 128:(m + 1) * 128, :], a7[:])
            allgather(blk3, G3, 512)

            # FC3 + bn8
            r3t = pfc.tile([128, 32, 64], dt.bfloat16, tag="r2")
            nc.sync.dma_start(r3t[:], G3[:].rearrange("(kt p) b -> p kt b", p=128))
            par8 = pfw.tile([125, 2], dt.float32, tag="par8")
            nc.sync.dma_start(par8[:], pt[8][:])
            l3v = l3_t[:].rearrange("(kt p) c -> p kt c", p=128)
            w3sb = pfw.tile([128, 32, 125], dt.bfloat16, tag="l3w")
            nc.sync.dma_start(w3sb[:], l3v[:])
            ps = ppsf.tile([128, 64], dt.float32, tag="psf")
            for kc in range(32):
                nc.tensor.matmul(ps[0:125, :], w3sb[:, kc, :], r3t[:, kc, :], start=kc == 0, stop=kc == 31)
            z8 = pfc.tile([125, 64], dt.float32, tag="z8")
            nc.vector.tensor_scalar(z8[:], ps[0:125, :], par8[:, 0:1], par8[:, 1:2], ALU.mult, ALU.add)
            nc.gpsimd.dma_start(blk4[:], z8[:])
            allgather(blk4, G4, 125)

            # gather logits -> [64, 1000] via PE transpose, then log_softmax
            idt = ppar.tile([128, 128], dt.float32, tag="ident")
            make_identity(nc, idt[:])
            lg = pfc.tile([64, 1000], dt.float32, tag="lg")
            for r in range(8):
                lt = pfc.tile([125, 64], dt.float32, tag="lt")
                nc.sync.dma_start(lt[:], G4[r * 125:(r + 1) * 125, :])
                tp = ppsf.tile([64, 128], dt.float32, tag="tp")
                nc.tensor.transpose(tp[0:64, 0:125], lt[:], idt[0:125, 0:125])
                nc.vector.tensor_copy(lg[:, r * 125:(r + 1) * 125], tp[0:64, 0:125])
            mx = pfc.tile([64, 1], dt.float32, tag="mx")
            nc.vector.reduce_max(mx[:], lg[:], axis=mybir.AxisListType.X)
            sh = pfc.tile([64, 1000], dt.float32, tag="sh")
            nc.vector.tensor_scalar(sh[:], lg[:], mx[:], None, ALU.subtract)
            ex = pfc.tile([64, 1000], dt.float32, tag="ex")
            nc.scalar.activation(ex[:], sh[:], AF.Exp)
            sm = pfc.tile([64, 1], dt.float32, tag="sm")
            nc.vector.reduce_sum(sm[:], ex[:], axis=mybir.AxisListType.X)
            ls = pfc.tile([64, 1], dt.float32, tag="ls")
            nc.scalar.activation(ls[:], sm[:], AF.Ln)
            osb = pfc.tile([64, 1000], dt.float32, tag="osb")
            nc.vector.tensor_scalar(osb[:], sh[:], ls[:], None, ALU.subtract)
            nc.sync.dma_start(out_t[:], osb[:])

            for k in reversed(list(stacks)):
                stacks.pop(k).close()

    nc.finalize()
    return nc


def get_nc(dump=()):
    key = tuple(sorted(dump))
    if key not in _CACHE:
        _CACHE[key] = build_nc(dump)
    return _CACHE[key]


def kernel(**inputs):
    from concourse.bass_utils import run_bass_kernel_spmd

    nc = get_nc(())
    in_maps = host_prep(inputs)
    res = run_bass_kernel_spmd(nc, in_maps, core_ids=list(range(NC)))
    out = np.asarray(res.results[0]["out"], np.float32)
    return out


# revision 12
# speedup vs baseline: 1.4905x; 1.3289x over previous
"""Bass/Trainium2 kernel for nn_AlexNetOWT_BN (binarized AlexNet, batch 64).

Strategy
--------
Data-parallel convolutions (8 images per core, conv weights replicated as
sign() values in fp8e4 -> exact integer arithmetic in fp32 PSUM), conv1 in a
hi/lo integer-split fp16 pair (exact PE arithmetic, ~3e-6 total error which is
far inside the min |bn-output| margin of ~2.4e-5), conv2 in fp8 DoubleRow
(K=256 per matmul: both ci tiles fused, halving PE stream cycles),
tensor-parallel FC layers (columns sharded 8-ways, activations exchanged via
AllGather), log_softmax computed redundantly on every core; core 0's [64,1000]
output is returned.

Conv biases and FC biases are folded into the BN shift on the host
(sign(inv*(x+cb)+shf) == sign(inv*x + (inv*cb+shf)), and max-pool commutes
with the per-channel constant add), so every post-matmul drain is a single
activation (or pool chain) reading PSUM directly.
"""

import os

import numpy as np
import ml_dtypes

NC = 8      # cores
B = 64      # batch
BC = 8      # images per core

F8NP = ml_dtypes.float8_e4m3fn
EPS = 1e-5

# conv layer tables (layers 3..5): 3x3 convs on 13x13, DoubleRow over ci pairs
#   pr: number of 256-channel ci pairs, n_mi: number of 128-wide co tiles
L3 = dict(name="c3", ci=576, co=1152, pr=3, n_mi=9, pool=False)
L4 = dict(name="c4", ci=1152, co=768, pr=5, n_mi=6, pool=False)
L5 = dict(name="c5", ci=768, co=256, pr=3, n_mi=2, pool=True)

CO2_T = [128, 128, 128, 128, 64]   # conv2 output-channel tiles


# ----------------------------------------------------------------------------
# host-side preprocessing
# ----------------------------------------------------------------------------

def _sgn(x):
    return np.sign(x).astype(np.float32)


def _convx_w_pack(w, n_pr):
    """[CO,CI,3,3] -> [n_mi, 128 p, 9 off, n_pr, 2 j, 128 c] fp8 DoubleRow layout."""
    co, ci, kh, kw = w.shape
    n_mi = co // 128
    sw = _sgn(w)
    out = np.zeros((n_mi, 128, 9, n_pr, 2, 128), np.float32)
    for mi in range(n_mi):
        for pr in range(n_pr):
            for j in range(2):
                ci0 = 256 * pr + 128 * j
                ciw = min(128, ci - ci0)
                if ciw <= 0:
                    continue
                blk = sw[mi * 128:(mi + 1) * 128, ci0:ci0 + ciw].reshape(128, ciw, 9).transpose(1, 2, 0)
                out[mi, :ciw, :, pr, j, :] = blk
    return np.ascontiguousarray(out.reshape(n_mi, 128, 9 * n_pr * 2 * 128)).astype(F8NP)


def _conv2_w_pack(w):
    """[576,192,5,5] -> [5 mi, 128 p, 25 off, 2 j, 128 c] fp8 DoubleRow layout."""
    sw = _sgn(w)  # [576, 192, 5, 5]
    out = np.zeros((5, 128, 25, 2, 128), np.float32)
    for mi, mw in enumerate(CO2_T):
        co0 = mi * 128
        for j in range(2):
            ci0, ciw = j * 128, min(128, 192 - j * 128)
            # [mw, ciw, 5, 5] -> [ciw(p), 25(off), mw(c)]
            blk = sw[co0:co0 + mw, ci0:ci0 + ciw].reshape(mw, ciw, 25).transpose(1, 2, 0)
            out[mi, :ciw, :, j, :mw] = blk
    return np.ascontiguousarray(out.reshape(5, 128, 25 * 2 * 128)).astype(F8NP)


def _im2col_c1(v):
    """[B,3,224,224] -> [384, B, 3025] fp16 (K order (ci,ky,kx), zero-padded)."""
    vp = np.pad(v, ((0, 0), (0, 0), (2, 2), (2, 2)))
    s = vp.strides
    w = np.lib.stride_tricks.as_strided(
        vp, (B, 3, 11, 11, 55, 55), (s[0], s[1], s[2], s[3], s[2] * 4, s[3] * 4))
    w = w.reshape(B, 363, 3025).transpose(1, 0, 2)
    out = np.zeros((384, B, 3025), np.float16)
    out[:363] = w
    return out


def _bn_params(cb, g, bb, m, v):
    """[inv, shf'] with the conv/linear bias folded into the shift."""
    inv = (g.astype(np.float32) / np.sqrt(v.astype(np.float32) + EPS)).astype(np.float32)
    shift = (bb.astype(np.float32) - m.astype(np.float32) * inv
             + cb.astype(np.float32) * inv).astype(np.float32)
    return np.ascontiguousarray(np.stack([inv, shift], 1))


def host_prep(inputs):
    d = inputs
    x = np.asarray(d["x"], np.float32)

    hi = np.rint(x * 256.0).astype(np.float32)
    assert np.abs(hi).max() <= 2040.0
    lo = np.rint((x - hi / 256.0) * float(2 ** 20)).astype(np.float32)
    assert np.abs(lo).max() <= 2048.0
    hi_c = _im2col_c1(hi)
    lo_c = _im2col_c1(lo)

    w1 = np.zeros((384, 192), np.float16)
    w1[:363] = _sgn(d["cw1"]).reshape(192, 363).T
    w1p = np.ascontiguousarray(w1.reshape(3, 128, 192))

    w2p = _conv2_w_pack(d["cw2"])
    w3p = _convx_w_pack(d["cw3"], L3["pr"])
    w4p = _convx_w_pack(d["cw4"], L4["pr"])
    w5p = _convx_w_pack(d["cw5"], L5["pr"])

    p1 = _bn_params(d["cb1"], d["g1"], d["bb1"], d["m1"], d["v1"])
    p2 = _bn_params(d["cb2"], d["g2"], d["bb2"], d["m2"], d["v2"])
    p3 = _bn_params(d["cb3"], d["g3"], d["bb3"], d["m3"], d["v3"])
    p4 = _bn_params(d["cb4"], d["g4"], d["bb4"], d["m4"], d["v4"])
    p5 = _bn_params(d["cb5"], d["g5"], d["bb5"], d["m5"], d["v5"])
    p6 = _bn_params(d["lb1"], d["g6"], d["bb6"], d["m6"], d["v6"])
    p7 = _bn_params(d["lb2"], d["g7"], d["bb7"], d["m7"], d["v7"])
    p8 = _bn_params(d["lb3"], d["g8"], d["bb8"], d["m8"], d["v8"])

    l1t = np.ascontiguousarray(_sgn(d["lw1"]).T).astype(ml_dtypes.bfloat16)   # [9216, 4096]
    l2t = np.ascontiguousarray(_sgn(d["lw2"]).T).astype(ml_dtypes.bfloat16)   # [4096, 4096]
    l3t = np.ascontiguousarray(_sgn(d["lw3"]).T).astype(ml_dtypes.bfloat16)   # [4096, 1000]

    in_maps = []
    for r in range(NC):
        sl = slice(r * BC, (r + 1) * BC)
        m = dict(
            rh=np.ascontiguousarray(hi_c[:, sl]).reshape(384, BC * 3025).astype(np.float16),
            rl=np.ascontiguousarray(lo_c[:, sl]).reshape(384, BC * 3025).astype(np.float16),
            w1p=w1p, w2p=w2p, w3p=w3p, w4p=w4p, w5p=w5p,
            p1=p1, p2=p2, p3=p3, p4=p4, p5=p5,
            p6=np.ascontiguousarray(p6[r * 512:(r + 1) * 512]),
            p7=np.ascontiguousarray(p7[r * 512:(r + 1) * 512]),
            p8=np.ascontiguousarray(p8[r * 125:(r + 1) * 125]),
            l1t=np.ascontiguousarray(l1t[:, r * 512:(r + 1) * 512]),
            l2t=np.ascontiguousarray(l2t[:, r * 512:(r + 1) * 512]),
            l3t=np.ascontiguousarray(l3t[:, r * 125:(r + 1) * 125]),
        )
        in_maps.append(m)
    return in_maps


# ----------------------------------------------------------------------------
# device program
# ----------------------------------------------------------------------------

_CACHE = {}


def build_nc(dump=(), single=False, reps=1):
    import concourse.bass as bass  # noqa: F401
    import concourse.mybir as mybir
    import concourse.tile as tile
    from concourse import bacc
    from concourse.masks import make_identity
    from contextlib import ExitStack

    dt = mybir.dt
    AF = mybir.ActivationFunctionType
    ALU = mybir.AluOpType
    DR = mybir.MatmulPerfMode.DoubleRow

    nc = bacc.Bacc(num_devices=NC)

    # ---- I/O ----
    rh_t = nc.dram_tensor("rh", [384, BC * 3025], dt.float16, kind="ExternalInput")
    rl_t = nc.dram_tensor("rl", [384, BC * 3025], dt.float16, kind="ExternalInput")
    w1_t = nc.dram_tensor("w1p", [3, 128, 192], dt.float16, kind="ExternalInput")
    w2_t = nc.dram_tensor("w2p", [5, 128, 25 * 2 * 128], dt.float8e4, kind="ExternalInput")
    wts = {}
    for L in (L3, L4, L5):
        wts[L["name"]] = nc.dram_tensor(
            "w%sp" % L["name"][1], [L["n_mi"], 128, 9 * L["pr"] * 2 * 128],
            dt.float8e4, kind="ExternalInput")
    pt = {}
    for i, c in zip(range(1, 9), (192, 576, 1152, 768, 256, 512, 512, 125)):
        pt[i] = nc.dram_tensor("p%d" % i, [c, 2], dt.float32, kind="ExternalInput")
    l1_t = nc.dram_tensor("l1t", [9216, 512], dt.bfloat16, kind="ExternalInput")
    l2_t = nc.dram_tensor("l2t", [4096, 512], dt.bfloat16, kind="ExternalInput")
    l3_t = nc.dram_tensor("l3t", [4096, 125], dt.bfloat16, kind="ExternalInput")
    out_t = nc.dram_tensor("out", [64, 1000], dt.float32, kind="ExternalOutput")

    RG = [list(range(NC))]

    def allgather(blk, G, rows):
        if single:
            nc.gpsimd.dma_start(G[0:rows], blk[:])
        else:
            nc.gpsimd.collective_compute("AllGather", mybir.AluOpType.bypass, replica_groups=RG,
                                         ins=[blk[:].opt()], outs=[G[:].opt()])

    with tile.TileContext(nc) as tc:
        stacks = {}

        cur_pfx = [""]

        def open_pool(key, **kw):
            key = cur_pfx[0] + key
            s = ExitStack()
            p = s.enter_context(tc.tile_pool(name=key, **kw))
            stacks[key] = s
            return p

        def close_pool(key):
            stacks.pop(cur_pfx[0] + key).close()

        for _rep in range(reps):
            _pfx = "r%d_" % _rep if reps > 1 else ""
            cur_pfx[0] = _pfx
            blk1 = nc.dram_tensor(_pfx + "blk1", [BC, 9216], dt.bfloat16)
            G1 = nc.dram_tensor(_pfx + "G1", [64, 9216], dt.bfloat16)
            blk2 = nc.dram_tensor(_pfx + "blk2", [512, 64], dt.bfloat16)
            G2 = nc.dram_tensor(_pfx + "G2", [4096, 64], dt.bfloat16)
            blk3 = nc.dram_tensor(_pfx + "blk3", [512, 64], dt.bfloat16)
            G3 = nc.dram_tensor(_pfx + "G3", [4096, 64], dt.bfloat16)
            blk4 = nc.dram_tensor(_pfx + "blk4", [125, 64], dt.float32)
            G4 = nc.dram_tensor(_pfx + "G4", [1000, 64], dt.float32)

            # ------------------------------------------------------------------
            # stage 1: conv1 (fp16 hi/lo) -> pool -> bn -> sign -> a1 (fp8 DR)
            # ------------------------------------------------------------------
            ppar = open_pool("par", bufs=1)
            pa1 = open_pool("a1", bufs=1)
            pr1 = open_pool("r1", bufs=2)
            pw1 = open_pool("w1", bufs=1)
            pc1 = open_pool("c1", bufs=2)
            pps1 = open_pool("ps1", bufs=3, space="PSUM")

            w1sb = pw1.tile([128, 3, 192], dt.float16)
            nc.sync.dma_start(w1sb[:], w1_t[:].rearrange("k p c -> p k c"))

            # conv2 input, DoubleRow layout: [p, j(ci half), b, y, x]
            a1dr = pa1.tile([128, 2, BC, 31, 32], dt.float8e4, name="a1dr")
            nc.vector.memset(a1dr[:], 0.0)
            par1s = []
            for m, (m0, mw) in enumerate(((0, 128), (128, 64))):
                par1 = ppar.tile([128, 2], dt.float32, tag="par1_%d" % m)
                nc.sync.dma_start(par1[0:mw], pt[1][m0:m0 + mw, :])
                par1s.append(par1)

            rhv = rh_t[:].rearrange("(kt p) n -> p kt n", p=128)
            rlv = rl_t[:].rearrange("(kt p) n -> p kt n", p=128)
            NT1 = [(i * 512, min(512, 3025 - i * 512)) for i in range(6)]
            for b in range(BC):
                rht = pr1.tile([128, 3, 3025], dt.float16, tag="rh")
                nc.gpsimd.dma_start(rht[:], rhv[:, :, b * 3025:(b + 1) * 3025])
                rlt = pr1.tile([128, 3, 3025], dt.float16, tag="rl")
                nc.gpsimd.dma_start(rlt[:], rlv[:, :, b * 3025:(b + 1) * 3025])
                c1i = [pc1.tile([128, 3025], dt.float32, tag="c1_%d" % m, name="c1_%d" % m) for m in range(2)]
                for m, (m0, mw) in enumerate(((0, 128), (128, 64))):
                    par1 = par1s[m]
                    for n0, nn in NT1:
                        ph = pps1.tile([128, 512], dt.float32, tag="ph")
                        pl = pps1.tile([128, 512], dt.float32, tag="pl")
                        for kt in range(3):
                            nc.tensor.matmul(ph[0:mw, 0:nn], w1sb[:, kt, m0:m0 + mw],
                                             rht[:, kt, n0:n0 + nn], start=kt == 0, stop=kt == 2)
                        for kt in range(3):
                            nc.tensor.matmul(pl[0:mw, 0:nn], w1sb[:, kt, m0:m0 + mw],
                                             rlt[:, kt, n0:n0 + nn], start=kt == 0, stop=kt == 2)
                        tlo = pc1.tile([128, 512], dt.float32, tag="tlo")
                        nc.vector.tensor_scalar(tlo[0:mw, 0:nn], pl[0:mw, 0:nn],
                                                float(2 ** -20), None, ALU.mult)
                        nc.vector.scalar_tensor_tensor(c1i[m][0:mw, n0:n0 + nn], ph[0:mw, 0:nn],
                                                       float(2 ** -8), tlo[0:mw, 0:nn], ALU.mult, ALU.add)
                    # maxpool 55->27 then bn+sign
                    v = c1i[m][0:mw].rearrange("p (y x) -> p y x", x=55)
                    ty = pc1.tile([128, 27, 55], dt.float32, tag="ty1")
                    nc.vector.tensor_tensor(ty[0:mw], v[:, 0:53:2, :], v[:, 1:54:2, :], ALU.max)
                    nc.vector.tensor_tensor(ty[0:mw], ty[0:mw], v[:, 2:55:2, :], ALU.max)
                    pld = pc1.tile([128, 27, 27], dt.float32, tag="pl1")
                    nc.vector.tensor_tensor(pld[0:mw], ty[0:mw, :, 0:53:2], ty[0:mw, :, 1:54:2], ALU.max)
                    nc.vector.tensor_tensor(pld[0:mw], pld[0:mw], ty[0:mw, :, 2:55:2], ALU.max)
                    nc.scalar.activation(a1dr[0:mw, m, b, 2:29, 2:29], pld[0:mw], AF.Sign,
                                         bias=par1[0:mw, 1:2], scale=par1[0:mw, 0:1])
            close_pool("ps1"); close_pool("c1"); close_pool("w1"); close_pool("r1")

            # ------------------------------------------------------------------
            # conv2: fp8 DoubleRow (K=256) -> a2x pair tiles [p, j, b2, 15, 32]
            # (2 images x-packed per plane: img s at cols 16s+1..16s+13)
            # ------------------------------------------------------------------
            pa2 = open_pool("a2", bufs=1)
            a2x = [pa2.tile([128, 2, 4, 15, 32], dt.float8e4, tag="a2_%d" % i, name="a2_%d" % i)
                   for i in range(L3["pr"])]
            for t in a2x:
                nc.vector.memset(t[:], 0.0)
            pw2 = open_pool("w2", bufs=2)
            pc2 = open_pool("cc2", bufs=4)
            pps2 = open_pool("ps2", bufs=1, space="PSUM")

            co0 = 0
            for mi, mw in enumerate(CO2_T):
                wsb = pw2.tile([128, 25, 2, 128], dt.float8e4, tag="w2")
                nc.sync.dma_start(wsb[:].rearrange("p k j c -> p (k j c)"), w2_t[mi])
                part = pw2.tile([128, 2], dt.float32, tag="par2")
                nc.sync.dma_start(part[0:mw], pt[2][co0:co0 + mw, :])
                inv = part[0:mw, 0:1]
                shf = part[0:mw, 1:2]
                oc = co0 // 128
                op = co0 % 128
                for g in range(2):
                    psA = [pps2.tile([128, 18, 27], dt.float32, tag="psA%d" % i, name="psA%d" % i) for i in range(4)]
                    psB = [pps2.tile([128, 9, 27], dt.float32, tag="psB%d" % i, name="psB%d" % i) for i in range(4)]
                    for off in range(25):
                        ky, kx = off // 5, off % 5
                        for i in range(4):
                            b = 4 * g + i
                            nc.tensor.matmul(psA[i][0:mw], wsb[:, off, :, 0:mw],
                                             a1dr[:, :, b, ky:ky + 18, kx:kx + 27],
                                             start=off == 0, stop=off == 24, perf_mode=DR)
                            nc.tensor.matmul(psB[i][0:mw], wsb[:, off, :, 0:mw],
                                             a1dr[:, :, b, 18 + ky:27 + ky, kx:kx + 27],
                                             start=off == 0, stop=off == 24, perf_mode=DR)
                    for i in range(4):
                        b = 4 * g + i
                        c2 = pc2.tile([128, 27, 27], dt.float32, tag="img2")
                        nc.vector.tensor_copy(c2[0:mw, 0:18], psA[i][0:mw])
                        nc.vector.tensor_copy(c2[0:mw, 18:27], psB[i][0:mw])
                        ty = pc2.tile([128, 13, 27], dt.float32, tag="ty2")
                        nc.vector.tensor_tensor(ty[0:mw], c2[0:mw, 0:25:2], c2[0:mw, 1:26:2], ALU.max)
                        nc.vector.tensor_tensor(ty[0:mw], ty[0:mw], c2[0:mw, 2:27:2], ALU.max)
                        pld = pc2.tile([128, 13, 13], dt.float32, tag="pool2")
                        nc.vector.tensor_tensor(pld[0:mw], ty[0:mw, :, 0:25:2], ty[0:mw, :, 1:26:2], ALU.max)
                        nc.vector.tensor_tensor(pld[0:mw], pld[0:mw], ty[0:mw, :, 2:27:2], ALU.max)
                        x0 = 16 * (b % 2) + 1
                        nc.scalar.activation(a2x[mi // 2][0:mw, mi % 2, b // 2, 1:14, x0:x0 + 13],
                                             pld[0:mw], AF.Sign, bias=shf, scale=inv)
                co0 += mw
            close_pool("ps2"); close_pool("cc2"); close_pool("w2")

            # ------------------------------------------------------------------
            # conv layers 3..5: fp8 DoubleRow over ci pairs, x-packed image
            # pairs (rhs [128, 2, 13, 29], N=377; out img s at psum cols 16s..)
            # ------------------------------------------------------------------
            def conv_layer(L, idx, ain, out_pr):
                n_pr, n_mi = L["pr"], L["n_mi"]
                name = L["name"]
                KT = 9 * n_pr
                if name == "c5":
                    pa = open_pool("a5", bufs=1)
                    aout = [pa.tile([128, BC, 36], dt.bfloat16, tag="a5_%d" % i, name="a5_%d" % i) for i in range(2)]
                else:
                    pa = open_pool("a" + str(idx), bufs=1)
                    aout = [pa.tile([128, 2, 4, 15, 32], dt.float8e4, tag="a%d_%d" % (idx, i), name="a%d_%d" % (idx, i))
                            for i in range(out_pr)]
                    for t in aout:
                        nc.vector.memset(t[:], 0.0)
                pw = open_pool("w" + name, bufs=2)
                pc = open_pool("cc" + name, bufs=4)
                pps = open_pool("ps" + name, bufs=6, space="PSUM")

                for mi in range(n_mi):
                    wsb = pw.tile([128, 9, n_pr, 2, 128], dt.float8e4, tag="w" + name)
                    nc.sync.dma_start(wsb[:].rearrange("p k r j c -> p (k r j c)"), wts[name][mi])
                    part = pw.tile([128, 2], dt.float32, tag="par" + name)
                    nc.sync.dma_start(part[:], pt[idx][mi * 128:(mi + 1) * 128, :])
                    inv = part[:, 0:1]
                    shf = part[:, 1:2]
                    for b2 in range(4):
                        ps = pps.tile([128, 13, 29], dt.float32, tag="ps" + name)
                        k = 0
                        for off in range(9):
                            ky, kx = off // 3, off % 3
                            for pr in range(n_pr):
                                nc.tensor.matmul(
                                    ps[:], wsb[:, off, pr, :, :],
                                    ain[pr][:, :, b2, ky:ky + 13, kx:kx + 29],
                                    start=k == 0, stop=k == KT - 1, perf_mode=DR)
                                k += 1
                        if name == "c5":
                            y5 = pc.tile([128, 13, 29], dt.float32, tag="img" + name)
                            nc.vector.tensor_copy(y5[:], ps[:])
                            for s in range(2):
                                b = 2 * b2 + s
                                x0 = 16 * s
                                ty = pc.tile([128, 6, 13], dt.float32, tag="ty" + name)
                                nc.vector.tensor_tensor(ty[:], y5[:, 0:11:2, x0:x0 + 13],
                                                        y5[:, 1:12:2, x0:x0 + 13], ALU.max)
                                nc.vector.tensor_tensor(ty[:], ty[:], y5[:, 2:13:2, x0:x0 + 13], ALU.max)
                                pld = pc.tile([128, 6, 6], dt.float32, tag="pool" + name)
                                nc.vector.tensor_tensor(pld[:], ty[:, :, 0:11:2],
                                                        ty[:, :, 1:12:2], ALU.max)
                                nc.vector.tensor_tensor(pld[:], pld[:], ty[:, :, 2:13:2], ALU.max)
                                nc.scalar.activation(
                                    aout[mi][:, b].rearrange("p (y x) -> p y x", x=6),
                                    pld[:], AF.Sign, bias=shf, scale=inv)
                        else:
                            for s in range(2):
                                x0 = 16 * s
                                nc.scalar.activation(
                                    aout[mi // 2][:, mi % 2, b2, 1:14, x0 + 1:x0 + 14],
                                    ps[:, :, x0:x0 + 13], AF.Sign, bias=shf, scale=inv)
                close_pool("ps" + name); close_pool("cc" + name); close_pool("w" + name)
                return aout

            a3x = conv_layer(L3, 3, a2x, L4["pr"])
            a4x = conv_layer(L4, 4, a3x, L5["pr"])
            a5t = conv_layer(L5, 5, a4x, 0)

            # ------------------------------------------------------------------
            # FC layers (tensor parallel) + log_softmax
            # ------------------------------------------------------------------
            pfc = open_pool("fc", bufs=2)
            pfw = open_pool("fw", bufs=2)
            ppsf = open_pool("psf", bufs=4, space="PSUM")

            # a5 -> blk1 [8, 9216] (b-major rows)
            b1v = blk1[:].rearrange("b (c hw) -> c b hw", hw=36)
            for ch in range(2):
                nc.gpsimd.dma_start(b1v[ch * 128:(ch + 1) * 128], a5t[ch][:])
            allgather(blk1, G1, BC)

            # FC1: rhs via transpose DMAs
            r1t = pfc.tile([128, 72, 64], dt.bfloat16, tag="r1")
            for kc in range(72):
                nc.sync.dma_start(r1t[:, kc, :], G1[0:64, kc * 128:(kc + 1) * 128], transpose=True)
            l1v = l1_t[:].rearrange("(kt p) c -> p kt c", p=128)
            for m in range(4):
                wsb = pfw.tile([128, 72, 128], dt.bfloat16, tag="l1w")
                nc.sync.dma_start(wsb[:], l1v[:, :, m * 128:(m + 1) * 128])
                par6 = pfw.tile([128, 2], dt.float32, tag="par6")
                nc.sync.dma_start(par6[:], pt[6][m * 128:(m + 1) * 128, :])
                ps = ppsf.tile([128, 64], dt.float32, tag="psf")
                for kc in range(72):
                    nc.tensor.matmul(ps[:], wsb[:, kc, :], r1t[:, kc, :], start=kc == 0, stop=kc == 71)
                a6 = pfc.tile([128, 64], dt.bfloat16, tag="a6")
                nc.scalar.activation(a6[:], ps[:], AF.Sign,
                                     bias=par6[:, 1:2], scale=par6[:, 0:1])
                nc.gpsimd.dma_start(blk2[m * 128:(m + 1) * 128, :], a6[:])
            allgather(blk2, G2, 512)

            # FC2
            r2t = pfc.tile([128, 32, 64], dt.bfloat16, tag="r2")
            nc.sync.dma_start(r2t[:], G2[:].rearrange("(kt p) b -> p kt b", p=128))
            l2v = l2_t[:].rearrange("(kt p) c -> p kt c", p=128)
            for m in range(4):
                wsb = pfw.tile([128, 32, 128], dt.bfloat16, tag="l2w")
                nc.sync.dma_start(wsb[:], l2v[:, :, m * 128:(m + 1) * 128])
                par7 = pfw.tile([128, 2], dt.float32, tag="par6")
                nc.sync.dma_start(par7[:], pt[7][m * 128:(m + 1) * 128, :])
                ps = ppsf.tile([128, 64], dt.float32, tag="psf")
                for kc in range(32):
                    nc.tensor.matmul(ps[:], wsb[:, kc, :], r2t[:, kc, :], start=kc == 0, stop=kc == 31)
                a7 = pfc.tile([128, 64], dt.bfloat16, tag="a6")
                nc.scalar.activation(a7[:], ps[:], AF.Sign,
                                     bias=par7[:, 1:2], scale=par7[:, 0:1])
                nc.gpsimd.dma_start(blk3[m * 128:(m + 1) * 128, :], a7[:])
            allgather(blk3, G3, 512)

            # FC3 + bn8
            r3t = pfc.tile([128, 32, 64], dt.bfloat16, tag="r2")
            nc.sync.dma_start(r3t[:], G3[:].rearrange("(kt p) b -> p kt b", p=128))
            par8 = pfw.tile([125, 2], dt.float32, tag="par8")
            nc.sync.dma_start(par8[:], pt[8][:])
            l3v = l3_t[:].rearrange("(kt p) c -> p kt c", p=128)
            w3sb = pfw.tile([128, 32, 125], dt.bfloat16, tag="l3w")
            nc.sync.dma_start(w3sb[:], l3v[:])
            ps = ppsf.tile([128, 64], dt.float32, tag="psf")
            for kc in range(32):
                nc.tensor.matmul(ps[0:125, :], w3sb[:, kc, :], r3t[:, kc, :], start=kc == 0, stop=kc == 31)
            z8 = pfc.tile([125, 64], dt.float32, tag="z8")
            nc.vector.tensor_scalar(z8[:], ps[0:125, :], par8[:, 0:1], par8[:, 1:2], ALU.mult, ALU.add)
            nc.gpsimd.dma_start(blk4[:], z8[:])
            allgather(blk4, G4, 125)

            # gather logits -> [64, 1000] via PE transpose, then log_softmax
            idt = ppar.tile([128, 128], dt.float32, tag="ident")
            make_identity(nc, idt[:])
            lg = pfc.tile([64, 1000], dt.float32, tag="lg")
            for r in range(8):
                lt = pfc.tile([125, 64], dt.float32, tag="lt")
                nc.sync.dma_start(lt[:], G4[r * 125:(r + 1) * 125, :])
                tp = ppsf.tile([64, 128], dt.float32, tag="tp")
                nc.tensor.transpose(tp[0:64, 0:125], lt[:], idt[0:125, 0:125])
                nc.vector.tensor_copy(lg[:, r * 125:(r + 1) * 125], tp[0:64, 0:125])
            mx = pfc.tile([64, 1], dt.float32, tag="mx")
            nc.vector.reduce_max(mx[:], lg[:], axis=mybir.AxisListType.X)
            sh = pfc.tile([64, 1000], dt.float32, tag="sh")
            nc.vector.tensor_scalar(sh[:], lg[:], mx[:], None, ALU.subtract)
            ex = pfc.tile([64, 1000], dt.float32, tag="ex")
            nc.scalar.activation(ex[:], sh[:], AF.Exp)
            sm = pfc.tile([64, 1], dt.float32, tag="sm")
            nc.vector.reduce_sum(sm[:], ex[:], axis=mybir.AxisListType.X)
            ls = pfc.tile([64, 1], dt.float32, tag="ls")
            nc.scalar.activation(ls[:], sm[:], AF.Ln)
            osb = pfc.tile([64, 1000], dt.float32, tag="osb")
            nc.vector.tensor_scalar(osb[:], sh[:], ls[:], None, ALU.subtract)
            nc.sync.dma_start(out_t[:], osb[:])

            for k in reversed(list(stacks)):
                stacks.pop(k).close()

    nc.finalize()
    return nc


def get_nc(dump=()):
    key = tuple(sorted(dump))
    if key not in _CACHE:
        _CACHE[key] = build_nc(dump)
    return _CACHE[key]


def kernel(**inputs):
    from concourse.bass_utils import run_bass_kernel_spmd

    nc = get_nc(())
    in_maps = host_prep(inputs)
    res = run_bass_kernel_spmd(nc, in_maps, core_ids=list(range(NC)))
    out = np.asarray(res.results[0]["out"], np.float32)
    return out


# revision 13
# speedup vs baseline: 2.1879x; 1.4679x over previous
"""Bass/Trainium2 kernel for nn_AlexNetOWT_BN (binarized AlexNet, batch 64).

Strategy
--------
Data-parallel convolutions (8 images per core, conv weights replicated as
sign() values in fp8e4 -> exact integer arithmetic in fp32 PSUM), conv1 in a
hi/lo integer-split fp16 pair (exact PE arithmetic, ~3e-6 total error which is
far inside the min |bn-output| margin of ~2.4e-5), conv2 in fp8 DoubleRow
(K=256 per matmul: both ci tiles fused, halving PE stream cycles),
tensor-parallel FC layers (columns sharded 8-ways, activations exchanged via
AllGather), log_softmax computed redundantly on every core; core 0's [64,1000]
output is returned.

Conv biases and FC biases are folded into the BN shift on the host
(sign(inv*(x+cb)+shf) == sign(inv*x + (inv*cb+shf)), and max-pool commutes
with the per-channel constant add), so every post-matmul drain is a single
activation (or pool chain) reading PSUM directly.
"""

import os

import numpy as np
import ml_dtypes

NC = 8      # cores
B = 64      # batch
BC = 8      # images per core

F8NP = ml_dtypes.float8_e4m3fn
EPS = 1e-5

# conv layer tables (layers 3..5): 3x3 convs on 13x13, DoubleRow over ci pairs
#   pr: number of 256-channel ci pairs, n_mi: number of 128-wide co tiles
L3 = dict(name="c3", ci=576, co=1152, pr=3, n_mi=9, pool=False)
L4 = dict(name="c4", ci=1152, co=768, pr=5, n_mi=6, pool=False)
L5 = dict(name="c5", ci=768, co=256, pr=3, n_mi=2, pool=True)

CO2_T = [128, 128, 128, 128, 64]   # conv2 output-channel tiles


# ----------------------------------------------------------------------------
# host-side preprocessing
# ----------------------------------------------------------------------------

def _sgn(x):
    return np.sign(x).astype(np.float32)


def _convx_w_pack(w, n_pr):
    """[CO,CI,3,3] -> [n_mi, 128 p, 9 off, n_pr, 2 j, 128 c] fp8 DoubleRow layout."""
    co, ci, kh, kw = w.shape
    n_mi = co // 128
    sw = _sgn(w)
    out = np.zeros((n_mi, 128, 9, n_pr, 2, 128), np.float32)
    for mi in range(n_mi):
        for pr in range(n_pr):
            for j in range(2):
                ci0 = 256 * pr + 128 * j
                ciw = min(128, ci - ci0)
                if ciw <= 0:
                    continue
                blk = sw[mi * 128:(mi + 1) * 128, ci0:ci0 + ciw].reshape(128, ciw, 9).transpose(1, 2, 0)
                out[mi, :ciw, :, pr, j, :] = blk
    return np.ascontiguousarray(out.reshape(n_mi, 128, 9 * n_pr * 2 * 128)).astype(F8NP)


def _conv2_w_pack(w):
    """[576,192,5,5] -> [5 mi, 128 p, 25 off, 2 j, 128 c] fp8 DoubleRow layout."""
    sw = _sgn(w)  # [576, 192, 5, 5]
    out = np.zeros((5, 128, 25, 2, 128), np.float32)
    for mi, mw in enumerate(CO2_T):
        co0 = mi * 128
        for j in range(2):
            ci0, ciw = j * 128, min(128, 192 - j * 128)
            # [mw, ciw, 5, 5] -> [ciw(p), 25(off), mw(c)]
            blk = sw[co0:co0 + mw, ci0:ci0 + ciw].reshape(mw, ciw, 25).transpose(1, 2, 0)
            out[mi, :ciw, :, j, :mw] = blk
    return np.ascontiguousarray(out.reshape(5, 128, 25 * 2 * 128)).astype(F8NP)


def _im2col_c1(v):
    """[B,3,224,224] -> [384, B, 3025] fp16 (K order (ci,ky,kx), zero-padded)."""
    vp = np.pad(v, ((0, 0), (0, 0), (2, 2), (2, 2)))
    s = vp.strides
    w = np.lib.stride_tricks.as_strided(
        vp, (B, 3, 11, 11, 55, 55), (s[0], s[1], s[2], s[3], s[2] * 4, s[3] * 4))
    w = w.reshape(B, 363, 3025).transpose(1, 0, 2)
    out = np.zeros((384, B, 3025), np.float16)
    out[:363] = w
    return out


def _bn_params(cb, g, bb, m, v):
    """[inv, shf'] with the conv/linear bias folded into the shift."""
    inv = (g.astype(np.float32) / np.sqrt(v.astype(np.float32) + EPS)).astype(np.float32)
    shift = (bb.astype(np.float32) - m.astype(np.float32) * inv
             + cb.astype(np.float32) * inv).astype(np.float32)
    return np.ascontiguousarray(np.stack([inv, shift], 1))


def host_prep(inputs):
    d = inputs
    x = np.asarray(d["x"], np.float32)

    hi = np.rint(x * 256.0).astype(np.float32)
    assert np.abs(hi).max() <= 2040.0
    lo = np.rint((x - hi / 256.0) * float(2 ** 20)).astype(np.float32)
    assert np.abs(lo).max() <= 2048.0
    hi_c = _im2col_c1(hi)
    lo_c = _im2col_c1(lo)

    w1 = np.zeros((384, 192), np.float16)
    w1[:363] = _sgn(d["cw1"]).reshape(192, 363).T
    w1p = np.ascontiguousarray(w1.reshape(3, 128, 192))

    w2p = _conv2_w_pack(d["cw2"])
    w3p = _convx_w_pack(d["cw3"], L3["pr"])
    w4p = _convx_w_pack(d["cw4"], L4["pr"])
    w5p = _convx_w_pack(d["cw5"], L5["pr"])

    p1 = _bn_params(d["cb1"], d["g1"], d["bb1"], d["m1"], d["v1"])
    p2 = _bn_params(d["cb2"], d["g2"], d["bb2"], d["m2"], d["v2"])
    p3 = _bn_params(d["cb3"], d["g3"], d["bb3"], d["m3"], d["v3"])
    p4 = _bn_params(d["cb4"], d["g4"], d["bb4"], d["m4"], d["v4"])
    p5 = _bn_params(d["cb5"], d["g5"], d["bb5"], d["m5"], d["v5"])
    p6 = _bn_params(d["lb1"], d["g6"], d["bb6"], d["m6"], d["v6"])
    p7 = _bn_params(d["lb2"], d["g7"], d["bb7"], d["m7"], d["v7"])
    p8 = _bn_params(d["lb3"], d["g8"], d["bb8"], d["m8"], d["v8"])

    l1t = np.ascontiguousarray(_sgn(d["lw1"]).T).astype(ml_dtypes.bfloat16)   # [9216, 4096]
    l2t = np.ascontiguousarray(_sgn(d["lw2"]).T).astype(ml_dtypes.bfloat16)   # [4096, 4096]
    l3t = np.ascontiguousarray(_sgn(d["lw3"]).T).astype(ml_dtypes.bfloat16)   # [4096, 1000]

    in_maps = []
    for r in range(NC):
        sl = slice(r * BC, (r + 1) * BC)
        m = dict(
            rh=np.ascontiguousarray(hi_c[:, sl]).reshape(384, BC * 3025).astype(np.float16),
            rl=np.ascontiguousarray(lo_c[:, sl]).reshape(384, BC * 3025).astype(np.float16),
            w1p=w1p, w2p=w2p, w3p=w3p, w4p=w4p, w5p=w5p,
            p1=p1, p2=p2, p3=p3, p4=p4, p5=p5,
            p6=np.ascontiguousarray(p6[r * 512:(r + 1) * 512]),
            p7=np.ascontiguousarray(p7[r * 512:(r + 1) * 512]),
            p8=np.ascontiguousarray(p8[r * 125:(r + 1) * 125]),
            l1t=np.ascontiguousarray(l1t[:, r * 512:(r + 1) * 512]),
            l2t=np.ascontiguousarray(l2t[:, r * 512:(r + 1) * 512]),
            l3t=np.ascontiguousarray(l3t[:, r * 125:(r + 1) * 125]),
        )
        in_maps.append(m)
    return in_maps


# ----------------------------------------------------------------------------
# device program
# ----------------------------------------------------------------------------

_CACHE = {}


def build_nc(dump=(), single=False, reps=1):
    import concourse.bass as bass  # noqa: F401
    import concourse.mybir as mybir
    import concourse.tile as tile
    from concourse import bacc
    from concourse.masks import make_identity
    from contextlib import ExitStack

    dt = mybir.dt
    AF = mybir.ActivationFunctionType
    ALU = mybir.AluOpType
    DR = mybir.MatmulPerfMode.DoubleRow

    nc = bacc.Bacc(num_devices=NC)

    # ---- I/O ----
    rh_t = nc.dram_tensor("rh", [384, BC * 3025], dt.float16, kind="ExternalInput")
    rl_t = nc.dram_tensor("rl", [384, BC * 3025], dt.float16, kind="ExternalInput")
    w1_t = nc.dram_tensor("w1p", [3, 128, 192], dt.float16, kind="ExternalInput")
    w2_t = nc.dram_tensor("w2p", [5, 128, 25 * 2 * 128], dt.float8e4, kind="ExternalInput")
    wts = {}
    for L in (L3, L4, L5):
        wts[L["name"]] = nc.dram_tensor(
            "w%sp" % L["name"][1], [L["n_mi"], 128, 9 * L["pr"] * 2 * 128],
            dt.float8e4, kind="ExternalInput")
    pt = {}
    for i, c in zip(range(1, 9), (192, 576, 1152, 768, 256, 512, 512, 125)):
        pt[i] = nc.dram_tensor("p%d" % i, [c, 2], dt.float32, kind="ExternalInput")
    l1_t = nc.dram_tensor("l1t", [9216, 512], dt.bfloat16, kind="ExternalInput")
    l2_t = nc.dram_tensor("l2t", [4096, 512], dt.bfloat16, kind="ExternalInput")
    l3_t = nc.dram_tensor("l3t", [4096, 125], dt.bfloat16, kind="ExternalInput")
    out_t = nc.dram_tensor("out", [64, 1000], dt.float32, kind="ExternalOutput")

    RG = [list(range(NC))]

    def allgather(blk, G, rows):
        if single:
            nc.gpsimd.dma_start(G[0:rows], blk[:])
        else:
            nc.gpsimd.collective_compute("AllGather", mybir.AluOpType.bypass, replica_groups=RG,
                                         ins=[blk[:].opt()], outs=[G[:].opt()])

    with tile.TileContext(nc) as tc:
        stacks = {}

        cur_pfx = [""]

        def open_pool(key, **kw):
            key = cur_pfx[0] + key
            s = ExitStack()
            p = s.enter_context(tc.tile_pool(name=key, **kw))
            stacks[key] = s
            return p

        def close_pool(key):
            stacks.pop(cur_pfx[0] + key).close()

        for _rep in range(reps):
            _pfx = "r%d_" % _rep if reps > 1 else ""
            cur_pfx[0] = _pfx
            blk1 = nc.dram_tensor(_pfx + "blk1", [BC, 9216], dt.bfloat16)
            G1 = nc.dram_tensor(_pfx + "G1", [64, 9216], dt.bfloat16)
            blk2 = nc.dram_tensor(_pfx + "blk2", [512, 64], dt.bfloat16)
            G2 = nc.dram_tensor(_pfx + "G2", [4096, 64], dt.bfloat16)
            blk3 = nc.dram_tensor(_pfx + "blk3", [512, 64], dt.bfloat16)
            G3 = nc.dram_tensor(_pfx + "G3", [4096, 64], dt.bfloat16)
            blk4 = nc.dram_tensor(_pfx + "blk4", [125, 64], dt.float32)
            G4 = nc.dram_tensor(_pfx + "G4", [1000, 64], dt.float32)

            # ------------------------------------------------------------------
            # stage 1: conv1 (fp16 hi/lo) -> pool -> bn -> sign -> a1 (fp8 DR)
            # ------------------------------------------------------------------
            ppar = open_pool("par", bufs=1)
            pa1 = open_pool("a1", bufs=1)
            pr1 = open_pool("r1", bufs=2)
            pw1 = open_pool("w1", bufs=1)
            pc1 = open_pool("c1", bufs=2)
            pps1 = open_pool("ps1", bufs=3, space="PSUM")

            w1sb = pw1.tile([128, 3, 192], dt.float16)
            nc.sync.dma_start(w1sb[:], w1_t[:].rearrange("k p c -> p k c"))

            # conv2 input, DoubleRow layout: [p, j(ci half), b, y, x]
            a1dr = pa1.tile([128, 2, BC, 31, 32], dt.float8e4, name="a1dr")
            nc.vector.memset(a1dr[:], 0.0)
            par1s = []
            for m, (m0, mw) in enumerate(((0, 128), (128, 64))):
                par1 = ppar.tile([128, 2], dt.float32, tag="par1_%d" % m)
                nc.sync.dma_start(par1[0:mw], pt[1][m0:m0 + mw, :])
                par1s.append(par1)

            rhv = rh_t[:].rearrange("(kt p) n -> p kt n", p=128)
            rlv = rl_t[:].rearrange("(kt p) n -> p kt n", p=128)
            NT1 = [(i * 512, min(512, 3025 - i * 512)) for i in range(6)]
            for b in range(BC):
                rht = pr1.tile([128, 3, 3025], dt.float16, tag="rh")
                nc.gpsimd.dma_start(rht[:], rhv[:, :, b * 3025:(b + 1) * 3025])
                rlt = pr1.tile([128, 3, 3025], dt.float16, tag="rl")
                nc.gpsimd.dma_start(rlt[:], rlv[:, :, b * 3025:(b + 1) * 3025])
                c1i = [pc1.tile([128, 3025], dt.float32, tag="c1_%d" % m, name="c1_%d" % m) for m in range(2)]
                for m, (m0, mw) in enumerate(((0, 128), (128, 64))):
                    par1 = par1s[m]
                    for n0, nn in NT1:
                        ph = pps1.tile([128, 512], dt.float32, tag="ph")
                        pl = pps1.tile([128, 512], dt.float32, tag="pl")
                        for kt in range(3):
                            nc.tensor.matmul(ph[0:mw, 0:nn], w1sb[:, kt, m0:m0 + mw],
                                             rht[:, kt, n0:n0 + nn], start=kt == 0, stop=kt == 2)
                        for kt in range(3):
                            nc.tensor.matmul(pl[0:mw, 0:nn], w1sb[:, kt, m0:m0 + mw],
                                             rlt[:, kt, n0:n0 + nn], start=kt == 0, stop=kt == 2)
                        tlo = pc1.tile([128, 512], dt.float32, tag="tlo")
                        nc.scalar.activation(tlo[0:mw, 0:nn], pl[0:mw, 0:nn],
                                             AF.Copy, scale=float(2 ** -20))
                        nc.vector.scalar_tensor_tensor(c1i[m][0:mw, n0:n0 + nn], ph[0:mw, 0:nn],
                                                       float(2 ** -8), tlo[0:mw, 0:nn], ALU.mult, ALU.add)
                    # maxpool 55->27 then bn+sign (row-pool on GpSimd, col on DVE)
                    v = c1i[m][0:mw].rearrange("p (y x) -> p y x", x=55)
                    ty = pc1.tile([128, 27, 55], dt.float32, tag="ty1")
                    nc.gpsimd.tensor_tensor(ty[0:mw], v[:, 0:53:2, :], v[:, 1:54:2, :], ALU.max)
                    nc.gpsimd.tensor_tensor(ty[0:mw], ty[0:mw], v[:, 2:55:2, :], ALU.max)
                    pld = pc1.tile([128, 27, 27], dt.float32, tag="pl1")
                    nc.vector.tensor_tensor(pld[0:mw], ty[0:mw, :, 0:53:2], ty[0:mw, :, 1:54:2], ALU.max)
                    nc.vector.tensor_tensor(pld[0:mw], pld[0:mw], ty[0:mw, :, 2:55:2], ALU.max)
                    nc.scalar.activation(a1dr[0:mw, m, b, 2:29, 2:29], pld[0:mw], AF.Sign,
                                         bias=par1[0:mw, 1:2], scale=par1[0:mw, 0:1])
            close_pool("ps1"); close_pool("c1"); close_pool("w1"); close_pool("r1")

            # ------------------------------------------------------------------
            # conv2: fp8 DoubleRow (K=256) -> a2x pair tiles [p, j, b2, 15, 32]
            # (2 images x-packed per plane: img s at cols 16s+1..16s+13)
            # ------------------------------------------------------------------
            pa2 = open_pool("a2", bufs=1)
            a2x = [pa2.tile([128, 2, 4, 15, 32], dt.float8e4, tag="a2_%d" % i, name="a2_%d" % i)
                   for i in range(L3["pr"])]
            for t in a2x:
                nc.vector.memset(t[:], 0.0)
            pw2 = open_pool("w2", bufs=2)
            pc2 = open_pool("cc2", bufs=4)
            pps2 = open_pool("ps2", bufs=1, space="PSUM")

            co0 = 0
            for mi, mw in enumerate(CO2_T):
                wsb = pw2.tile([128, 25, 2, 128], dt.float8e4, tag="w2")
                nc.sync.dma_start(wsb[:].rearrange("p k j c -> p (k j c)"), w2_t[mi])
                part = pw2.tile([128, 2], dt.float32, tag="par2")
                nc.sync.dma_start(part[0:mw], pt[2][co0:co0 + mw, :])
                inv = part[0:mw, 0:1]
                shf = part[0:mw, 1:2]
                oc = co0 // 128
                op = co0 % 128
                for g in range(2):
                    psA = [pps2.tile([128, 18, 27], dt.float32, tag="psA%d" % i, name="psA%d" % i) for i in range(4)]
                    psB = [pps2.tile([128, 9, 27], dt.float32, tag="psB%d" % i, name="psB%d" % i) for i in range(4)]
                    for off in range(25):
                        ky, kx = off // 5, off % 5
                        for i in range(4):
                            b = 4 * g + i
                            nc.tensor.matmul(psA[i][0:mw], wsb[:, off, :, 0:mw],
                                             a1dr[:, :, b, ky:ky + 18, kx:kx + 27],
                                             start=off == 0, stop=off == 24, perf_mode=DR)
                            nc.tensor.matmul(psB[i][0:mw], wsb[:, off, :, 0:mw],
                                             a1dr[:, :, b, 18 + ky:27 + ky, kx:kx + 27],
                                             start=off == 0, stop=off == 24, perf_mode=DR)
                    for i in range(4):
                        b = 4 * g + i
                        c2 = pc2.tile([128, 27, 27], dt.float32, tag="img2")
                        nc.vector.tensor_copy(c2[0:mw, 0:18], psA[i][0:mw])
                        nc.vector.tensor_copy(c2[0:mw, 18:27], psB[i][0:mw])
                        ty = pc2.tile([128, 13, 27], dt.float32, tag="ty2")
                        nc.vector.tensor_tensor(ty[0:mw], c2[0:mw, 0:25:2], c2[0:mw, 1:26:2], ALU.max)
                        nc.vector.tensor_tensor(ty[0:mw], ty[0:mw], c2[0:mw, 2:27:2], ALU.max)
                        pld = pc2.tile([128, 13, 13], dt.float32, tag="pool2")
                        nc.vector.tensor_tensor(pld[0:mw], ty[0:mw, :, 0:25:2], ty[0:mw, :, 1:26:2], ALU.max)
                        nc.vector.tensor_tensor(pld[0:mw], pld[0:mw], ty[0:mw, :, 2:27:2], ALU.max)
                        x0 = 16 * (b % 2) + 1
                        nc.scalar.activation(a2x[mi // 2][0:mw, mi % 2, b // 2, 1:14, x0:x0 + 13],
                                             pld[0:mw], AF.Sign, bias=shf, scale=inv)
                co0 += mw
            close_pool("ps2"); close_pool("cc2"); close_pool("w2")

            # ------------------------------------------------------------------
            # conv layers 3..5: fp8 DoubleRow over ci pairs, x-packed image
            # pairs (rhs [128, 2, 13, 29], N=377; out img s at psum cols 16s..)
            # ------------------------------------------------------------------
            def conv_layer(L, idx, ain, out_pr):
                n_pr, n_mi = L["pr"], L["n_mi"]
                name = L["name"]
                KT = 9 * n_pr
                if name == "c5":
                    pa = open_pool("a5", bufs=1)
                    aout = [pa.tile([128, BC, 36], dt.bfloat16, tag="a5_%d" % i, name="a5_%d" % i) for i in range(2)]
                else:
                    pa = open_pool("a" + str(idx), bufs=1)
                    aout = [pa.tile([128, 2, 4, 15, 32], dt.float8e4, tag="a%d_%d" % (idx, i), name="a%d_%d" % (idx, i))
                            for i in range(out_pr)]
                    for t in aout:
                        nc.vector.memset(t[:], 0.0)
                pw = open_pool("w" + name, bufs=2)
                pc = open_pool("cc" + name, bufs=4)
                pps = open_pool("ps" + name, bufs=6, space="PSUM")

                for mi in range(n_mi):
                    wsb = pw.tile([128, 9, n_pr, 2, 128], dt.float8e4, tag="w" + name)
                    nc.sync.dma_start(wsb[:].rearrange("p k r j c -> p (k r j c)"), wts[name][mi])
                    part = pw.tile([128, 2], dt.float32, tag="par" + name)
                    nc.sync.dma_start(part[:], pt[idx][mi * 128:(mi + 1) * 128, :])
                    inv = part[:, 0:1]
                    shf = part[:, 1:2]
                    for b2 in range(4):
                        ps = pps.tile([128, 13, 29], dt.float32, tag="ps" + name)
                        k = 0
                        for off in range(9):
                            ky, kx = off // 3, off % 3
                            for pr in range(n_pr):
                                nc.tensor.matmul(
                                    ps[:], wsb[:, off, pr, :, :],
                                    ain[pr][:, :, b2, ky:ky + 13, kx:kx + 29],
                                    start=k == 0, stop=k == KT - 1, perf_mode=DR)
                                k += 1
                        if name == "c5":
                            y5 = pc.tile([128, 13, 29], dt.float32, tag="img" + name)
                            nc.vector.tensor_copy(y5[:], ps[:])
                            for s in range(2):
                                b = 2 * b2 + s
                                x0 = 16 * s
                                ty = pc.tile([128, 6, 13], dt.float32, tag="ty" + name)
                                nc.vector.tensor_tensor(ty[:], y5[:, 0:11:2, x0:x0 + 13],
                                                        y5[:, 1:12:2, x0:x0 + 13], ALU.max)
                                nc.vector.tensor_tensor(ty[:], ty[:], y5[:, 2:13:2, x0:x0 + 13], ALU.max)
                                pld = pc.tile([128, 6, 6], dt.float32, tag="pool" + name)
                                nc.vector.tensor_tensor(pld[:], ty[:, :, 0:11:2],
                                                        ty[:, :, 1:12:2], ALU.max)
                                nc.vector.tensor_tensor(pld[:], pld[:], ty[:, :, 2:13:2], ALU.max)
                                nc.scalar.activation(
                                    aout[mi][:, b].rearrange("p (y x) -> p y x", x=6),
                                    pld[:], AF.Sign, bias=shf, scale=inv)
                        else:
                            for s in range(2):
                                x0 = 16 * s
                                nc.scalar.activation(
                                    aout[mi // 2][:, mi % 2, b2, 1:14, x0 + 1:x0 + 14],
                                    ps[:, :, x0:x0 + 13], AF.Sign, bias=shf, scale=inv)
                close_pool("ps" + name); close_pool("cc" + name); close_pool("w" + name)
                return aout

            a3x = conv_layer(L3, 3, a2x, L4["pr"])
            a4x = conv_layer(L4, 4, a3x, L5["pr"])
            a5t = conv_layer(L5, 5, a4x, 0)

            # ------------------------------------------------------------------
            # FC layers (tensor parallel) + log_softmax
            # ------------------------------------------------------------------
            pfc = open_pool("fc", bufs=2)
            pfw = open_pool("fw", bufs=2)
            ppsf = open_pool("psf", bufs=4, space="PSUM")

            # a5 -> blk1 [8, 9216] (b-major rows)
            b1v = blk1[:].rearrange("b (c hw) -> c b hw", hw=36)
            for ch in range(2):
                nc.gpsimd.dma_start(b1v[ch * 128:(ch + 1) * 128], a5t[ch][:])
            allgather(blk1, G1, BC)

            # FC1: rhs via transpose DMAs
            r1t = pfc.tile([128, 72, 64], dt.bfloat16, tag="r1")
            for kc in range(72):
                nc.sync.dma_start(r1t[:, kc, :], G1[0:64, kc * 128:(kc + 1) * 128], transpose=True)
            l1v = l1_t[:].rearrange("(kt p) c -> p kt c", p=128)
            for m in range(4):
                wsb = pfw.tile([128, 72, 128], dt.bfloat16, tag="l1w")
                nc.sync.dma_start(wsb[:], l1v[:, :, m * 128:(m + 1) * 128])
                par6 = pfw.tile([128, 2], dt.float32, tag="par6")
                nc.sync.dma_start(par6[:], pt[6][m * 128:(m + 1) * 128, :])
                ps = ppsf.tile([128, 64], dt.float32, tag="psf")
                for kc in range(72):
                    nc.tensor.matmul(ps[:], wsb[:, kc, :], r1t[:, kc, :], start=kc == 0, stop=kc == 71)
                a6 = pfc.tile([128, 64], dt.bfloat16, tag="a6")
                nc.scalar.activation(a6[:], ps[:], AF.Sign,
                                     bias=par6[:, 1:2], scale=par6[:, 0:1])
                nc.gpsimd.dma_start(blk2[m * 128:(m + 1) * 128, :], a6[:])
            allgather(blk2, G2, 512)

            # FC2
            r2t = pfc.tile([128, 32, 64], dt.bfloat16, tag="r2")
            nc.sync.dma_start(r2t[:], G2[:].rearrange("(kt p) b -> p kt b", p=128))
            l2v = l2_t[:].rearrange("(kt p) c -> p kt c", p=128)
            for m in range(4):
                wsb = pfw.tile([128, 32, 128], dt.bfloat16, tag="l2w")
                nc.sync.dma_start(wsb[:], l2v[:, :, m * 128:(m + 1) * 128])
                par7 = pfw.tile([128, 2], dt.float32, tag="par6")
                nc.sync.dma_start(par7[:], pt[7][m * 128:(m + 1) * 128, :])
                ps = ppsf.tile([128, 64], dt.float32, tag="psf")
                for kc in range(32):
                    nc.tensor.matmul(ps[:], wsb[:, kc, :], r2t[:, kc, :], start=kc == 0, stop=kc == 31)
                a7 = pfc.tile([128, 64], dt.bfloat16, tag="a6")
                nc.scalar.activation(a7[:], ps[:], AF.Sign,
                                     bias=par7[:, 1:2], scale=par7[:, 0:1])
                nc.gpsimd.dma_start(blk3[m * 128:(m + 1) * 128, :], a7[:])
            allgather(blk3, G3, 512)

            # FC3 + bn8
            r3t = pfc.tile([128, 32, 64], dt.bfloat16, tag="r2")
            nc.sync.dma_start(r3t[:], G3[:].rearrange("(kt p) b -> p kt b", p=128))
            par8 = pfw.tile([125, 2], dt.float32, tag="par8")
            nc.sync.dma_start(par8[:], pt[8][:])
            l3v = l3_t[:].rearrange("(kt p) c -> p kt c", p=128)
            w3sb = pfw.tile([128, 32, 125], dt.bfloat16, tag="l3w")
            nc.sync.dma_start(w3sb[:], l3v[:])
            ps = ppsf.tile([128, 64], dt.float32, tag="psf")
            for kc in range(32):
                nc.tensor.matmul(ps[0:125, :], w3sb[:, kc, :], r3t[:, kc, :], start=kc == 0, stop=kc == 31)
            z8 = pfc.tile([125, 64], dt.float32, tag="z8")
            nc.vector.tensor_scalar(z8[:], ps[0:125, :], par8[:, 0:1], par8[:, 1:2], ALU.mult, ALU.add)
            nc.gpsimd.dma_start(blk4[:], z8[:])
            allgather(blk4, G4, 125)

            # gather logits -> [64, 1000] via PE transpose, then log_softmax
            idt = ppar.tile([128, 128], dt.float32, tag="ident")
            make_identity(nc, idt[:])
            lg = pfc.tile([64, 1000], dt.float32, tag="lg")
            for r in range(8):
                lt = pfc.tile([125, 64], dt.float32, tag="lt")
                nc.sync.dma_start(lt[:], G4[r * 125:(r + 1) * 125, :])
                tp = ppsf.tile([64, 128], dt.float32, tag="tp")
                nc.tensor.transpose(tp[0:64, 0:125], lt[:], idt[0:125, 0:125])
                nc.vector.tensor_copy(lg[:, r * 125:(r + 1) * 125], tp[0:64, 0:125])
            mx = pfc.tile([64, 1], dt.float32, tag="mx")
            nc.vector.reduce_max(mx[:], lg[:], axis=mybir.AxisListType.X)
            sh = pfc.tile([64, 1000], dt.float32, tag="sh")
            nc.vector.tensor_scalar(sh[:], lg[:], mx[:], None, ALU.subtract)
            ex = pfc.tile([64, 1000], dt.float32, tag="ex")
            nc.scalar.activation(ex[:], sh[:], AF.Exp)
            sm = pfc.tile([64, 1], dt.float32, tag="sm")
            nc.vector.reduce_sum(sm[:], ex[:], axis=mybir.AxisListType.X)
            ls = pfc.tile([64, 1], dt.float32, tag="ls")
            nc.scalar.activation(ls[:], sm[:], AF.Ln)
            osb = pfc.tile([64, 1000], dt.float32, tag="osb")
            nc.vector.tensor_scalar(osb[:], sh[:], ls[:], None, ALU.subtract)
            nc.sync.dma_start(out_t[:], osb[:])

            for k in reversed(list(stacks)):
                stacks.pop(k).close()

    nc.finalize()
    return nc


def get_nc(dump=()):
    key = tuple(sorted(dump))
    if key not in _CACHE:
        _CACHE[key] = build_nc(dump)
    return _CACHE[key]


def kernel(**inputs):
    from concourse.bass_utils import run_bass_kernel_spmd

    nc = get_nc(())
    in_maps = host_prep(inputs)
    res = run_bass_kernel_spmd(nc, in_maps, core_ids=list(range(NC)))
    out = np.asarray(res.results[0]["out"], np.float32)
    return out


# revision 16
# speedup vs baseline: 2.2777x; 1.0411x over previous
"""Bass/Trainium2 kernel for nn_AlexNetOWT_BN (binarized AlexNet, batch 64).

Strategy
--------
Data-parallel convolutions (8 images per core, conv weights replicated as
sign() values in fp8e4 -> exact integer arithmetic in fp32 PSUM), conv1 in a
hi/lo integer-split fp16 pair (exact PE arithmetic, ~3e-6 total error which is
far inside the min |bn-output| margin of ~2.4e-5), conv2 in fp8 DoubleRow
(K=256 per matmul: both ci tiles fused, halving PE stream cycles),
tensor-parallel FC layers (columns sharded 8-ways, activations exchanged via
AllGather), log_softmax computed redundantly on every core; core 0's [64,1000]
output is returned.

Conv biases and FC biases are folded into the BN shift on the host
(sign(inv*(x+cb)+shf) == sign(inv*x + (inv*cb+shf)), and max-pool commutes
with the per-channel constant add), so every post-matmul drain is a single
activation (or pool chain) reading PSUM directly.
"""

import os

import numpy as np
import ml_dtypes

NC = 8      # cores
B = 64      # batch
BC = 8      # images per core

F8NP = ml_dtypes.float8_e4m3fn
EPS = 1e-5

# conv layer tables (layers 3..5): 3x3 convs on 13x13, DoubleRow over ci pairs
#   pr: number of 256-channel ci pairs, n_mi: number of 128-wide co tiles
L3 = dict(name="c3", ci=576, co=1152, pr=3, n_mi=9, pool=False)
L4 = dict(name="c4", ci=1152, co=768, pr=5, n_mi=6, pool=False)
L5 = dict(name="c5", ci=768, co=256, pr=3, n_mi=2, pool=True)

CO2_T = [128, 128, 128, 128, 64]   # conv2 output-channel tiles


# ----------------------------------------------------------------------------
# host-side preprocessing
# ----------------------------------------------------------------------------

def _sgn(x):
    return np.sign(x).astype(np.float32)


def _convx_w_pack(w, n_pr):
    """[CO,CI,3,3] -> [n_mi, 128 p, 9 off, n_pr, 2 j, 128 c] fp8 DoubleRow layout."""
    co, ci, kh, kw = w.shape
    n_mi = co // 128
    sw = _sgn(w)
    out = np.zeros((n_mi, 128, 9, n_pr, 2, 128), np.float32)
    for mi in range(n_mi):
        for pr in range(n_pr):
            for j in range(2):
                ci0 = 256 * pr + 128 * j
                ciw = min(128, ci - ci0)
                if ciw <= 0:
                    continue
                blk = sw[mi * 128:(mi + 1) * 128, ci0:ci0 + ciw].reshape(128, ciw, 9).transpose(1, 2, 0)
                out[mi, :ciw, :, pr, j, :] = blk
    return np.ascontiguousarray(out.reshape(n_mi, 128, 9 * n_pr * 2 * 128)).astype(F8NP)


def _conv2_w_pack(w):
    """[576,192,5,5] -> [5 mi, 128 p, 25 off, 2 j, 128 c] fp8 DoubleRow layout."""
    sw = _sgn(w)  # [576, 192, 5, 5]
    out = np.zeros((5, 128, 25, 2, 128), np.float32)
    for mi, mw in enumerate(CO2_T):
        co0 = mi * 128
        for j in range(2):
            ci0, ciw = j * 128, min(128, 192 - j * 128)
            # [mw, ciw, 5, 5] -> [ciw(p), 25(off), mw(c)]
            blk = sw[co0:co0 + mw, ci0:ci0 + ciw].reshape(mw, ciw, 25).transpose(1, 2, 0)
            out[mi, :ciw, :, j, :mw] = blk
    return np.ascontiguousarray(out.reshape(5, 128, 25 * 2 * 128)).astype(F8NP)


def _im2col_c1(v):
    """[B,3,224,224] -> [384, B, 3025] fp16 (K order (ci,ky,kx), zero-padded)."""
    vp = np.pad(v, ((0, 0), (0, 0), (2, 2), (2, 2)))
    s = vp.strides
    w = np.lib.stride_tricks.as_strided(
        vp, (B, 3, 11, 11, 55, 55), (s[0], s[1], s[2], s[3], s[2] * 4, s[3] * 4))
    w = w.reshape(B, 363, 3025).transpose(1, 0, 2)
    out = np.zeros((384, B, 3025), np.float16)
    out[:363] = w
    return out


def _bn_params(cb, g, bb, m, v):
    """[inv, shf'] with the conv/linear bias folded into the shift."""
    inv = (g.astype(np.float32) / np.sqrt(v.astype(np.float32) + EPS)).astype(np.float32)
    shift = (bb.astype(np.float32) - m.astype(np.float32) * inv
             + cb.astype(np.float32) * inv).astype(np.float32)
    return np.ascontiguousarray(np.stack([inv, shift], 1))


def host_prep(inputs):
    d = inputs
    x = np.asarray(d["x"], np.float32)

    hi = np.rint(x * 256.0).astype(np.float32)
    assert np.abs(hi).max() <= 2040.0
    lo = np.rint((x - hi / 256.0) * float(2 ** 20)).astype(np.float32)
    assert np.abs(lo).max() <= 2048.0
    hi_c = _im2col_c1(hi)
    lo_c = _im2col_c1(lo)

    w1 = np.zeros((384, 192), np.float16)
    w1[:363] = _sgn(d["cw1"]).reshape(192, 363).T
    w1p = np.ascontiguousarray(w1.reshape(3, 128, 192))

    w2p = _conv2_w_pack(d["cw2"])
    w3p = _convx_w_pack(d["cw3"], L3["pr"])
    w4p = _convx_w_pack(d["cw4"], L4["pr"])
    w5p = _convx_w_pack(d["cw5"], L5["pr"])

    p1 = _bn_params(d["cb1"], d["g1"], d["bb1"], d["m1"], d["v1"])
    p2 = _bn_params(d["cb2"], d["g2"], d["bb2"], d["m2"], d["v2"])
    p3 = _bn_params(d["cb3"], d["g3"], d["bb3"], d["m3"], d["v3"])
    p4 = _bn_params(d["cb4"], d["g4"], d["bb4"], d["m4"], d["v4"])
    p5 = _bn_params(d["cb5"], d["g5"], d["bb5"], d["m5"], d["v5"])
    p6 = _bn_params(d["lb1"], d["g6"], d["bb6"], d["m6"], d["v6"])
    p7 = _bn_params(d["lb2"], d["g7"], d["bb7"], d["m7"], d["v7"])
    p8 = _bn_params(d["lb3"], d["g8"], d["bb8"], d["m8"], d["v8"])

    l1t = np.ascontiguousarray(_sgn(d["lw1"]).T).astype(ml_dtypes.bfloat16)   # [9216, 4096]
    l2t = np.ascontiguousarray(_sgn(d["lw2"]).T).astype(ml_dtypes.bfloat16)   # [4096, 4096]
    l3t = np.ascontiguousarray(_sgn(d["lw3"]).T).astype(ml_dtypes.bfloat16)   # [4096, 1000]

    in_maps = []
    for r in range(NC):
        sl = slice(r * BC, (r + 1) * BC)
        m = dict(
            rh=np.ascontiguousarray(hi_c[:, sl]).reshape(384, BC * 3025).astype(np.float16),
            rl=np.ascontiguousarray(lo_c[:, sl]).reshape(384, BC * 3025).astype(np.float16),
            w1p=w1p, w2p=w2p, w3p=w3p, w4p=w4p, w5p=w5p,
            p1=p1, p2=p2, p3=p3, p4=p4, p5=p5,
            p6=np.ascontiguousarray(p6[r * 512:(r + 1) * 512]),
            p7=np.ascontiguousarray(p7[r * 512:(r + 1) * 512]),
            p8=np.ascontiguousarray(p8[r * 125:(r + 1) * 125]),
            l1t=np.ascontiguousarray(l1t[:, r * 512:(r + 1) * 512]),
            l2t=np.ascontiguousarray(l2t[:, r * 512:(r + 1) * 512]),
            l3t=np.ascontiguousarray(l3t[:, r * 125:(r + 1) * 125]),
        )
        in_maps.append(m)
    return in_maps


# ----------------------------------------------------------------------------
# device program
# ----------------------------------------------------------------------------

_CACHE = {}


def build_nc(dump=(), single=False, reps=1):
    import concourse.bass as bass  # noqa: F401
    import concourse.mybir as mybir
    import concourse.tile as tile
    from concourse import bacc
    from concourse.masks import make_identity
    from contextlib import ExitStack

    dt = mybir.dt
    AF = mybir.ActivationFunctionType
    ALU = mybir.AluOpType
    DR = mybir.MatmulPerfMode.DoubleRow
    from concourse.ap import AP as _AP

    def _win(base, extra_off, *dims):
        """Overlapping-window AP: keep base's partition dim, replace free dims."""
        pairs = [[int(p[0]), int(p[1])] for p in base.ap]
        ap = [pairs[0]] + [[s_, n_] for s_, n_ in dims]
        return _AP(base.tensor, base.offset + extra_off, ap)

    nc = bacc.Bacc(num_devices=NC)

    # ---- I/O ----
    rh_t = nc.dram_tensor("rh", [384, BC * 3025], dt.float16, kind="ExternalInput")
    rl_t = nc.dram_tensor("rl", [384, BC * 3025], dt.float16, kind="ExternalInput")
    w1_t = nc.dram_tensor("w1p", [3, 128, 192], dt.float16, kind="ExternalInput")
    w2_t = nc.dram_tensor("w2p", [5, 128, 25 * 2 * 128], dt.float8e4, kind="ExternalInput")
    wts = {}
    for L in (L3, L4, L5):
        wts[L["name"]] = nc.dram_tensor(
            "w%sp" % L["name"][1], [L["n_mi"], 128, 9 * L["pr"] * 2 * 128],
            dt.float8e4, kind="ExternalInput")
    pt = {}
    for i, c in zip(range(1, 9), (192, 576, 1152, 768, 256, 512, 512, 125)):
        pt[i] = nc.dram_tensor("p%d" % i, [c, 2], dt.float32, kind="ExternalInput")
    l1_t = nc.dram_tensor("l1t", [9216, 512], dt.bfloat16, kind="ExternalInput")
    l2_t = nc.dram_tensor("l2t", [4096, 512], dt.bfloat16, kind="ExternalInput")
    l3_t = nc.dram_tensor("l3t", [4096, 125], dt.bfloat16, kind="ExternalInput")
    out_t = nc.dram_tensor("out", [64, 1000], dt.float32, kind="ExternalOutput")

    RG = [list(range(NC))]

    def allgather(blk, G, rows):
        if single:
            nc.gpsimd.dma_start(G[0:rows], blk[:])
        else:
            nc.gpsimd.collective_compute("AllGather", mybir.AluOpType.bypass, replica_groups=RG,
                                         ins=[blk[:].opt()], outs=[G[:].opt()])

    with tile.TileContext(nc) as tc:
        stacks = {}

        cur_pfx = [""]

        def open_pool(key, **kw):
            key = cur_pfx[0] + key
            s = ExitStack()
            p = s.enter_context(tc.tile_pool(name=key, **kw))
            stacks[key] = s
            return p

        def close_pool(key):
            stacks.pop(cur_pfx[0] + key).close()

        for _rep in range(reps):
            _pfx = "r%d_" % _rep if reps > 1 else ""
            cur_pfx[0] = _pfx
            blk1 = nc.dram_tensor(_pfx + "blk1", [BC, 9216], dt.bfloat16)
            G1 = nc.dram_tensor(_pfx + "G1", [64, 9216], dt.bfloat16)
            blk2 = nc.dram_tensor(_pfx + "blk2", [512, 64], dt.bfloat16)
            G2 = nc.dram_tensor(_pfx + "G2", [4096, 64], dt.bfloat16)
            blk3 = nc.dram_tensor(_pfx + "blk3", [512, 64], dt.bfloat16)
            G3 = nc.dram_tensor(_pfx + "G3", [4096, 64], dt.bfloat16)
            blk4 = nc.dram_tensor(_pfx + "blk4", [125, 64], dt.float32)
            G4 = nc.dram_tensor(_pfx + "G4", [1000, 64], dt.float32)

            # ------------------------------------------------------------------
            # stage 1: conv1 (fp16 hi/lo) -> pool -> bn -> sign -> a1 (fp8 DR)
            # ------------------------------------------------------------------
            ppar = open_pool("par", bufs=1)
            pa1 = open_pool("a1", bufs=1)
            pr1 = open_pool("r1", bufs=2)
            pw1 = open_pool("w1", bufs=1)
            pc1 = open_pool("c1", bufs=2)
            pps1 = open_pool("ps1", bufs=3, space="PSUM")

            w1sb = pw1.tile([128, 3, 192], dt.float16)
            nc.sync.dma_start(w1sb[:], w1_t[:].rearrange("k p c -> p k c"))

            # conv2 input, DoubleRow layout: [p, j(ci half), b, y, x]
            a1dr = pa1.tile([128, 2, BC, 31, 32], dt.float8e4, name="a1dr")
            nc.gpsimd.memset(a1dr[:], 0.0)
            par1s = []
            for m, (m0, mw) in enumerate(((0, 128), (128, 64))):
                par1 = ppar.tile([128, 2], dt.float32, tag="par1_%d" % m)
                nc.sync.dma_start(par1[0:mw], pt[1][m0:m0 + mw, :])
                par1s.append(par1)

            rhv = rh_t[:].rearrange("(kt p) n -> p kt n", p=128)
            rlv = rl_t[:].rearrange("(kt p) n -> p kt n", p=128)
            NT1 = [(i * 512, min(512, 3025 - i * 512)) for i in range(6)]
            for b in range(BC):
                rht = pr1.tile([128, 3, 3025], dt.float16, tag="rh")
                nc.gpsimd.dma_start(rht[:], rhv[:, :, b * 3025:(b + 1) * 3025])
                rlt = pr1.tile([128, 3, 3025], dt.float16, tag="rl")
                nc.gpsimd.dma_start(rlt[:], rlv[:, :, b * 3025:(b + 1) * 3025])
                c1i = [pc1.tile([128, 3025], dt.float32, tag="c1_%d" % m, name="c1_%d" % m) for m in range(2)]
                for m, (m0, mw) in enumerate(((0, 128), (128, 64))):
                    par1 = par1s[m]
                    for n0, nn in NT1:
                        ph = pps1.tile([128, 512], dt.float32, tag="ph")
                        pl = pps1.tile([128, 512], dt.float32, tag="pl")
                        for kt in range(3):
                            nc.tensor.matmul(ph[0:mw, 0:nn], w1sb[:, kt, m0:m0 + mw],
                                             rht[:, kt, n0:n0 + nn], start=kt == 0, stop=kt == 2)
                        for kt in range(3):
                            nc.tensor.matmul(pl[0:mw, 0:nn], w1sb[:, kt, m0:m0 + mw],
                                             rlt[:, kt, n0:n0 + nn], start=kt == 0, stop=kt == 2)
                        tlo = pc1.tile([128, 512], dt.float32, tag="tlo")
                        nc.scalar.activation(tlo[0:mw, 0:nn], pl[0:mw, 0:nn],
                                             AF.Copy, scale=float(2 ** -20))
                        nc.vector.scalar_tensor_tensor(c1i[m][0:mw, n0:n0 + nn], ph[0:mw, 0:nn],
                                                       float(2 ** -8), tlo[0:mw, 0:nn], ALU.mult, ALU.add)
                    # maxpool 55->27 (rows then cols)
                    v = c1i[m][0:mw].rearrange("p (y x) -> p y x", x=55)
                    ty = pc1.tile([128, 27, 55], dt.float32, tag="ty1")
                    nc.vector.tensor_tensor(ty[0:mw], v[:, 0:53:2, :], v[:, 1:54:2, :], ALU.max)
                    nc.vector.tensor_tensor(ty[0:mw], ty[0:mw], v[:, 2:55:2, :], ALU.max)
                    pld = pc1.tile([128, 27, 27], dt.float32, tag="pl1")
                    nc.vector.tensor_tensor(pld[0:mw], ty[0:mw, :, 0:53:2], ty[0:mw, :, 1:54:2], ALU.max)
                    nc.vector.tensor_tensor(pld[0:mw], pld[0:mw], ty[0:mw, :, 2:55:2], ALU.max)
                    nc.scalar.activation(a1dr[0:mw, m, b, 2:29, 2:29], pld[0:mw], AF.Sign,
                                         bias=par1[0:mw, 1:2], scale=par1[0:mw, 0:1])
            close_pool("ps1"); close_pool("c1"); close_pool("w1"); close_pool("r1")

            # ------------------------------------------------------------------
            # conv2: fp8 DoubleRow (K=256) -> a2x pair tiles [p, j, b2, 15, 32]
            # (2 images x-packed per plane: img s at cols 16s+1..16s+13)
            # ------------------------------------------------------------------
            pa2 = open_pool("a2", bufs=1)
            a2x = [pa2.tile([128, 2, 4, 15, 32], dt.float8e4, tag="a2_%d" % i, name="a2_%d" % i)
                   for i in range(L3["pr"])]
            for t in a2x:
                nc.gpsimd.memset(t[:], 0.0)
            pw2 = open_pool("w2", bufs=2)
            pc2 = open_pool("cc2", bufs=4)
            pps2 = open_pool("ps2", bufs=1, space="PSUM")

            co0 = 0
            for mi, mw in enumerate(CO2_T):
                wsb = pw2.tile([128, 25, 2, 128], dt.float8e4, tag="w2")
                nc.sync.dma_start(wsb[:].rearrange("p k j c -> p (k j c)"), w2_t[mi])
                part = pw2.tile([128, 2], dt.float32, tag="par2")
                nc.sync.dma_start(part[0:mw], pt[2][co0:co0 + mw, :])
                inv = part[0:mw, 0:1]
                shf = part[0:mw, 1:2]
                oc = co0 // 128
                op = co0 % 128
                for g in range(2):
                    psA = [pps2.tile([128, 18, 27], dt.float32, tag="psA%d" % i, name="psA%d" % i) for i in range(4)]
                    psB = [pps2.tile([128, 9, 27], dt.float32, tag="psB%d" % i, name="psB%d" % i) for i in range(4)]
                    for off in range(25):
                        ky, kx = off // 5, off % 5
                        for i in range(4):
                            b = 4 * g + i
                            nc.tensor.matmul(psA[i][0:mw], wsb[:, off, :, 0:mw],
                                             a1dr[:, :, b, ky:ky + 18, kx:kx + 27],
                                             start=off == 0, stop=off == 24, perf_mode=DR)
                            nc.tensor.matmul(psB[i][0:mw], wsb[:, off, :, 0:mw],
                                             a1dr[:, :, b, 18 + ky:27 + ky, kx:kx + 27],
                                             start=off == 0, stop=off == 24, perf_mode=DR)
                    for i in range(4):
                        b = 4 * g + i
                        c2 = pc2.tile([128, 27, 27], dt.float32, tag="img2")
                        nc.scalar.activation(c2[0:mw, 0:18], psA[i][0:mw], AF.Copy)
                        nc.scalar.activation(c2[0:mw, 18:27], psB[i][0:mw], AF.Copy)
                        ty = pc2.tile([128, 13, 27], dt.float32, tag="ty2")
                        nc.vector.tensor_tensor(ty[0:mw], c2[0:mw, 0:25:2], c2[0:mw, 1:26:2], ALU.max)
                        nc.vector.tensor_tensor(ty[0:mw], ty[0:mw], c2[0:mw, 2:27:2], ALU.max)
                        pld = pc2.tile([128, 13, 13], dt.float32, tag="pool2")
                        nc.vector.tensor_tensor(pld[0:mw], ty[0:mw, :, 0:25:2], ty[0:mw, :, 1:26:2], ALU.max)
                        nc.vector.tensor_tensor(pld[0:mw], pld[0:mw], ty[0:mw, :, 2:27:2], ALU.max)
                        x0 = 16 * (b % 2) + 1
                        nc.scalar.activation(a2x[mi // 2][0:mw, mi % 2, b // 2, 1:14, x0:x0 + 13],
                                             pld[0:mw], AF.Sign, bias=shf, scale=inv)
                co0 += mw
            close_pool("ps2"); close_pool("cc2"); close_pool("w2")

            # ------------------------------------------------------------------
            # conv layers 3..5: fp8 DoubleRow over ci pairs, x-packed image
            # pairs (rhs [128, 2, 13, 29], N=377; out img s at psum cols 16s..)
            # ------------------------------------------------------------------
            def conv_layer(L, idx, ain, out_pr):
                n_pr, n_mi = L["pr"], L["n_mi"]
                name = L["name"]
                KT = 9 * n_pr
                if name == "c5":
                    pa = open_pool("a5", bufs=1)
                    aout = [pa.tile([128, BC, 36], dt.bfloat16, tag="a5_%d" % i, name="a5_%d" % i) for i in range(2)]
                else:
                    pa = open_pool("a" + str(idx), bufs=1)
                    aout = [pa.tile([128, 2, 4, 15, 32], dt.float8e4, tag="a%d_%d" % (idx, i), name="a%d_%d" % (idx, i))
                            for i in range(out_pr)]
                    for t in aout:
                        nc.gpsimd.memset(t[:], 0.0)
                pw = open_pool("w" + name, bufs=2)
                pc = open_pool("cc" + name, bufs=4)
                pps = open_pool("ps" + name, bufs=6, space="PSUM")

                for mi in range(n_mi):
                    wsb = pw.tile([128, 9, n_pr, 2, 128], dt.float8e4, tag="w" + name)
                    nc.sync.dma_start(wsb[:].rearrange("p k r j c -> p (k r j c)"), wts[name][mi])
                    part = pw.tile([128, 2], dt.float32, tag="par" + name)
                    nc.sync.dma_start(part[:], pt[idx][mi * 128:(mi + 1) * 128, :])
                    inv = part[:, 0:1]
                    shf = part[:, 1:2]
                    for b2 in range(4):
                        ps = pps.tile([128, 13, 29], dt.float32, tag="ps" + name)
                        k = 0
                        for off in range(9):
                            ky, kx = off // 3, off % 3
                            for pr in range(n_pr):
                                nc.tensor.matmul(
                                    ps[:], wsb[:, off, pr, :, :],
                                    ain[pr][:, :, b2, ky:ky + 13, kx:kx + 29],
                                    start=k == 0, stop=k == KT - 1, perf_mode=DR)
                                k += 1
                        if name == "c5":
                            y5 = pc.tile([128, 13, 29], dt.float32, tag="img" + name)
                            nc.scalar.activation(y5[:], ps[:], AF.Copy)
                            for s in range(2):
                                b = 2 * b2 + s
                                x0 = 16 * s
                                ty = pc.tile([128, 6, 13], dt.float32, tag="ty" + name)
                                nc.vector.tensor_tensor(ty[:], y5[:, 0:11:2, x0:x0 + 13],
                                                        y5[:, 1:12:2, x0:x0 + 13], ALU.max)
                                nc.vector.tensor_tensor(ty[:], ty[:], y5[:, 2:13:2, x0:x0 + 13], ALU.max)
                                pld = pc.tile([128, 6, 6], dt.float32, tag="pool" + name)
                                nc.vector.tensor_tensor(pld[:], ty[:, :, 0:11:2],
                                                        ty[:, :, 1:12:2], ALU.max)
                                nc.vector.tensor_tensor(pld[:], pld[:], ty[:, :, 2:13:2], ALU.max)
                                nc.scalar.activation(
                                    aout[mi][:, b].rearrange("p (y x) -> p y x", x=6),
                                    pld[:], AF.Sign, bias=shf, scale=inv)
                        else:
                            for s in range(2):
                                x0 = 16 * s
                                nc.scalar.activation(
                                    aout[mi // 2][:, mi % 2, b2, 1:14, x0 + 1:x0 + 14],
                                    ps[:, :, x0:x0 + 13], AF.Sign, bias=shf, scale=inv)
                close_pool("ps" + name); close_pool("cc" + name); close_pool("w" + name)
                return aout

            a3x = conv_layer(L3, 3, a2x, L4["pr"])
            a4x = conv_layer(L4, 4, a3x, L5["pr"])
            a5t = conv_layer(L5, 5, a4x, 0)

            # ------------------------------------------------------------------
            # FC layers (tensor parallel) + log_softmax
            # ------------------------------------------------------------------
            pfc = open_pool("fc", bufs=2)
            pfw = open_pool("fw", bufs=2)
            ppsf = open_pool("psf", bufs=4, space="PSUM")

            # a5 -> blk1 [8, 9216] (b-major rows)
            b1v = blk1[:].rearrange("b (c hw) -> c b hw", hw=36)
            for ch in range(2):
                nc.gpsimd.dma_start(b1v[ch * 128:(ch + 1) * 128], a5t[ch][:])
            allgather(blk1, G1, BC)

            # FC1: rhs via transpose DMAs
            r1t = pfc.tile([128, 72, 64], dt.bfloat16, tag="r1")
            for kc in range(72):
                nc.sync.dma_start(r1t[:, kc, :], G1[0:64, kc * 128:(kc + 1) * 128], transpose=True)
            l1v = l1_t[:].rearrange("(kt p) c -> p kt c", p=128)
            for m in range(4):
                wsb = pfw.tile([128, 72, 128], dt.bfloat16, tag="l1w")
                nc.sync.dma_start(wsb[:], l1v[:, :, m * 128:(m + 1) * 128])
                par6 = pfw.tile([128, 2], dt.float32, tag="par6")
                nc.sync.dma_start(par6[:], pt[6][m * 128:(m + 1) * 128, :])
                ps = ppsf.tile([128, 64], dt.float32, tag="psf")
                for kc in range(72):
                    nc.tensor.matmul(ps[:], wsb[:, kc, :], r1t[:, kc, :], start=kc == 0, stop=kc == 71)
                a6 = pfc.tile([128, 64], dt.bfloat16, tag="a6")
                nc.scalar.activation(a6[:], ps[:], AF.Sign,
                                     bias=par6[:, 1:2], scale=par6[:, 0:1])
                nc.gpsimd.dma_start(blk2[m * 128:(m + 1) * 128, :], a6[:])
            allgather(blk2, G2, 512)

            # FC2
            r2t = pfc.tile([128, 32, 64], dt.bfloat16, tag="r2")
            nc.sync.dma_start(r2t[:], G2[:].rearrange("(kt p) b -> p kt b", p=128))
            l2v = l2_t[:].rearrange("(kt p) c -> p kt c", p=128)
            for m in range(4):
                wsb = pfw.tile([128, 32, 128], dt.bfloat16, tag="l2w")
                nc.sync.dma_start(wsb[:], l2v[:, :, m * 128:(m + 1) * 128])
                par7 = pfw.tile([128, 2], dt.float32, tag="par6")
                nc.sync.dma_start(par7[:], pt[7][m * 128:(m + 1) * 128, :])
                ps = ppsf.tile([128, 64], dt.float32, tag="psf")
                for kc in range(32):
                    nc.tensor.matmul(ps[:], wsb[:, kc, :], r2t[:, kc, :], start=kc == 0, stop=kc == 31)
                a7 = pfc.tile([128, 64], dt.bfloat16, tag="a6")
                nc.scalar.activation(a7[:], ps[:], AF.Sign,
                                     bias=par7[:, 1:2], scale=par7[:, 0:1])
                nc.gpsimd.dma_start(blk3[m * 128:(m + 1) * 128, :], a7[:])
            allgather(blk3, G3, 512)

            # FC3 + bn8
            r3t = pfc.tile([128, 32, 64], dt.bfloat16, tag="r2")
            nc.sync.dma_start(r3t[:], G3[:].rearrange("(kt p) b -> p kt b", p=128))
            par8 = pfw.tile([125, 2], dt.float32, tag="par8")
            nc.sync.dma_start(par8[:], pt[8][:])
            l3v = l3_t[:].rearrange("(kt p) c -> p kt c", p=128)
            w3sb = pfw.tile([128, 32, 125], dt.bfloat16, tag="l3w")
            nc.sync.dma_start(w3sb[:], l3v[:])
            ps = ppsf.tile([128, 64], dt.float32, tag="psf")
            for kc in range(32):
                nc.tensor.matmul(ps[0:125, :], w3sb[:, kc, :], r3t[:, kc, :], start=kc == 0, stop=kc == 31)
            z8 = pfc.tile([125, 64], dt.float32, tag="z8")
            nc.vector.tensor_scalar(z8[:], ps[0:125, :], par8[:, 0:1], par8[:, 1:2], ALU.mult, ALU.add)
            nc.gpsimd.dma_start(blk4[:], z8[:])
            allgather(blk4, G4, 125)

            # gather logits -> [64, 1000] via PE transpose, then log_softmax
            idt = ppar.tile([128, 128], dt.float32, tag="ident")
            make_identity(nc, idt[:])
            lg = pfc.tile([64, 1000], dt.float32, tag="lg")
            for r in range(8):
                lt = pfc.tile([125, 64], dt.float32, tag="lt")
                nc.sync.dma_start(lt[:], G4[r * 125:(r + 1) * 125, :])
                tp = ppsf.tile([64, 128], dt.float32, tag="tp")
                nc.tensor.transpose(tp[0:64, 0:125], lt[:], idt[0:125, 0:125])
                nc.vector.tensor_copy(lg[:, r * 125:(r + 1) * 125], tp[0:64, 0:125])
            mx = pfc.tile([64, 1], dt.float32, tag="mx")
            nc.vector.reduce_max(mx[:], lg[:], axis=mybir.AxisListType.X)
            sh = pfc.tile([64, 1000], dt.float32, tag="sh")
            nc.vector.tensor_scalar(sh[:], lg[:], mx[:], None, ALU.subtract)
            ex = pfc.tile([64, 1000], dt.float32, tag="ex")
            nc.scalar.activation(ex[:], sh[:], AF.Exp)
            sm = pfc.tile([64, 1], dt.float32, tag="sm")
            nc.vector.reduce_sum(sm[:], ex[:], axis=mybir.AxisListType.X)
            ls = pfc.tile([64, 1], dt.float32, tag="ls")
            nc.scalar.activation(ls[:], sm[:], AF.Ln)
            osb = pfc.tile([64, 1000], dt.float32, tag="osb")
            nc.vector.tensor_scalar(osb[:], sh[:], ls[:], None, ALU.subtract)
            nc.sync.dma_start(out_t[:], osb[:])

            for k in reversed(list(stacks)):
                stacks.pop(k).close()

    nc.finalize()
    return nc


def get_nc(dump=()):
    key = tuple(sorted(dump))
    if key not in _CACHE:
        _CACHE[key] = build_nc(dump)
    return _CACHE[key]


def kernel(**inputs):
    from concourse.bass_utils import run_bass_kernel_spmd

    nc = get_nc(())
    in_maps = host_prep(inputs)
    res = run_bass_kernel_spmd(nc, in_maps, core_ids=list(range(NC)))
    out = np.asarray(res.results[0]["out"], np.float32)
    return out
